# revision 44
# baseline (speedup 1.0000x reference)
"""Trainium2 Bass kernel for a 3-layer SAGE+GCN GNN on 50k nodes / 800k edges,
sharded across 8 NeuronCores.

Strategy (v2):
  - Nodes sharded into 8 contiguous ranges; edges assigned to the core that
    owns their dst node (host sorts edges by dst tile / src region).
  - Per conv, inputs are pre-projected to 128-dim fp16 "table" rows
    ([node, 256] = sage half | gcn half); every layer's table is built from
    the core's own shard and AllGathered (layer 1 included — no replicated
    full-graph projection).
  - Aggregation: indirect-DMA gather of each dst-tile's edge rows into SBUF,
    then one-hot matmul segment-sum on TensorE (M[e,d] = (dst[e]==d) * w_e
    built on VectorE; w_e folds the SAGE 1/deg or GCN norm).
  - idx16 gather indices are persistent in SBUF (loaded once), removing
    per-gather DMA dispatches.
  - Everything node-indexed on-chip is feature-major ([feat, node]); psum
    evacuations ride the Activation engine; residual adds on VectorE in fp16.

Measured-on-HW facts that shaped this (timing probes, 2026-08):
  - dma_gather is ~100% of device time; spreading gathers round-robin over
    num_swdge_queues=4 (queue_num=) gave ~3x on the gather phase. One queue
    runs ~60 GB/s effective; 4 queues ~119 GB/s (close to the HBM
    random-256B-row wall).
  - A single gather call is capped by the 1024-entry descriptor ring
    (CAPK=8 chunks x 128 rows works; larger crashes NRT).
  - Pool/GPSIMD ALU ops cost ~3 us each on HW — never offload elementwise
    work there (POOL_M stays 0).
  - Real AllGather (3.2MB in / 25.6MB out, 8 cores) is ~70-90 us, 4x faster
    than the cost model claims; barriers around collectives are unnecessary
    (Tile deps suffice) and removed.
  - fp16 tables/weights (vs bf16) cost nothing and cut rel err ~5x.
  - num_idxs need not be a multiple of 128: trimming each cell's final
    gather call to the true 16-rounded row count drops the ~12% pad
    descriptors (requires the startup memsets of the gather pool buffers —
    unwritten slots are zero-m'd but NaN*0=NaN would poison the psum).
  - In-cell src-address sorting and single_packet=0 measured neutral: the
    gather wall is descriptor-processing rate, not DRAM row locality.
"""

import os
import numpy as np

P = 128
NCORES = 8

LAST_EXEC_NS = None
LAST_TRACE = None


# ----------------------------------------------------------------------------
# host-side preprocessing
# ----------------------------------------------------------------------------

CAPK = int(os.environ.get("GNN_CAPK", "8"))    # chunks per dma_gather call
GW = int(os.environ.get("GNN_GW", "1"))        # dst tiles per one-hot group


def _edge_streams(src, dst, w_edge, n, shard, nt, tw, perm, bnd):
    """Per-core gather/M-build streams for one edge set, organized for
    nc.gpsimd.dma_gather: per (dst-group g of tw nodes, src-region r)
    gathers of NV[g][r] valid rows (idx-0 padded to a cross-core-uniform
    count). dst-local offsets (dl) are relative to the group base
    (0..tw-1, exact in fp16 for tw<=2048).

    Regions are defined in PERMUTED (chunk-major) table space via `bnd`
    boundaries so each region's table slice is produced by its own
    chunked AllGather (perm[n] = permuted row of node n).

    Returns per-core (idx16 [128, SIC], dw [128, 2*SK]) plus layout lists.
    """
    nreg = len(bnd) - 1
    ngr = (nt * P + tw - 1) // tw
    percore = []
    counts = np.zeros((NCORES, ngr, nreg), np.int64)
    for c in range(NCORES):
        lo, hi = c * shard, (c + 1) * shard
        m = (dst >= lo) & (dst < hi)
        s_c, d_c, w_c = perm[src[m]], dst[m] - lo, w_edge[m]
        reg_c = np.searchsorted(bnd, s_c, side="right") - 1
        # by (group, region), then ascending src address within the cell —
        # the in-cell order is free (dl/wv follow it) and sorted addresses
        # give the DMA engines DRAM row/bank locality on the random reads
        order = np.lexsort((s_c, reg_c, d_c // tw))
        s_c, d_c, w_c, reg_c = s_c[order], d_c[order], w_c[order], reg_c[order]
        key = (d_c // tw) * nreg + reg_c
        bounds = np.searchsorted(key, np.arange(ngr * nreg + 1))
        counts[c] = (bounds[1:] - bounds[:-1]).reshape(ngr, nreg)
        percore.append((s_c, d_c, w_c, bounds))
    NV = counts.max(axis=0)                       # [ngr, nreg] max real count
    K = np.maximum((NV + P - 1) // P, 1)          # chunks per (g, r)
    # CV: true gather row count per cell, 16-rounded — the final gather call
    # of a cell is trimmed to this (pad slots beyond it are never fetched;
    # their m columns are zero so stale SBUF data cannot contribute)
    CV = np.minimum((NV + 15) // 16 * 16, K * P)
    NV = K * P                                    # all slots valid (idx-0 pads)
    Ktot = K.sum(axis=1)                          # chunks per group
    SK = int(Ktot.sum())
    coff = np.zeros((ngr, nreg), np.int64)
    ioff = np.zeros((ngr, nreg), np.int64)
    acc_c = 0
    acc_i = 0
    for t in range(ngr):
        for r in range(nreg):
            coff[t, r] = acc_c
            ioff[t, r] = acc_i
            acc_c += K[t, r]
            acc_i += K[t, r] * 8
    SIC = int(acc_i)

    outs = []
    for c in range(NCORES):
        s_c, d_c, w_c, bounds = percore[c]
        idx16 = np.zeros((P, SIC), np.int16)
        dl = np.full((P, SK), -1.0, np.float32)
        wv = np.zeros((P, SK), np.float32)
        for t in range(ngr):
            for r in range(nreg):
                b0, b1 = bounds[t * nreg + r], bounds[t * nreg + r + 1]
                cnt = b1 - b0
                kr = int(K[t, r])
                slots = kr * P
                buf_i = np.zeros(slots, np.int32)   # idx-0 pads (always write)
                buf_i[:cnt] = s_c[b0:b1] - bnd[r]
                buf_d = np.full(slots, -1.0, np.float32)
                buf_d[:cnt] = (d_c[b0:b1] - t * tw).astype(np.float32)
                buf_w = np.zeros(slots, np.float32)
                buf_w[:cnt] = w_c[b0:b1]
                # idx16: wrapped over 16 partitions, replicated x8
                cols = kr * 8
                wrap = buf_i.reshape(cols, 16).T.astype(np.int16)  # [16, cols]
                io = int(ioff[t, r])
                idx16[:, io:io + cols] = np.tile(wrap, (8, 1))
                # dl/wv: slot i -> partition i%128, chunk coff+i//128
                co = int(coff[t, r])
                dl[:, co:co + kr] = buf_d.reshape(kr, P).T
                wv[:, co:co + kr] = buf_w.reshape(kr, P).T
        # fp16: dl values 0..tw-1 are exact; wv rounding is ~5e-4 rel
        dw = np.concatenate([dl, wv], axis=1).astype(np.float16)
        outs.append((idx16, dw))
    meta_es = dict(K=K.tolist(), NV=NV.tolist(), CV=CV.tolist(),
                   Ktot=[int(x) for x in Ktot],
                   coff=coff.tolist(), ioff=ioff.tolist(),
                   SK=SK, SIC=SIC, nreg=nreg, ngr=ngr)
    return outs, meta_es


def _prep(inputs):
    inp = {k: np.asarray(v) for k, v in inputs.items()}
    x = inp["x"].astype(np.float32)
    n, din = x.shape
    assert din == P
    shard = n // NCORES
    nt = (shard + P - 1) // P

    src = inp["edge_index"][0].astype(np.int64)
    dst = inp["edge_index"][1].astype(np.int64)
    srca = inp["edge_index_aux"][0].astype(np.int64)
    dsta = inp["edge_index_aux"][1].astype(np.int64)

    deg = np.zeros(n, np.float32)
    np.add.at(deg, dst, 1.0)
    recip_deg = (1.0 / np.maximum(deg, 1.0)).astype(np.float32)
    dega = np.zeros(n, np.float32)
    np.add.at(dega, dsta, 1.0)
    deg_hat = dega + 1.0
    rs = (1.0 / np.sqrt(deg_hat)).astype(np.float32)

    # chunk-major permuted table layout: core-local rows [0, q0) of every
    # core land in region 0 (permuted row c*q0 + i), the rest in region 1 —
    # each region is the output of its own chunked AllGather, so next-layer
    # region-0 gathers start while region 1 is still collecting
    ntr0 = int(os.environ.get("GNN_NTR0", "25"))   # tiles per core in region 0
    q0 = min(ntr0 * P, shard)
    c_ = np.arange(n) // shard
    i_ = np.arange(n) % shard
    perm = np.where(i_ < q0, c_ * q0 + i_,
                    NCORES * q0 + c_ * (shard - q0) + (i_ - q0))
    bnd = np.array([0, NCORES * q0, n], np.int64)
    assert max(bnd[1], n - bnd[1]) < 32768  # int16 gather offsets

    sage_streams, es_s = _edge_streams(src, dst, recip_deg[dst], n, shard, nt,
                                       GW * P, perm, bnd)
    # GCN self-loop == a (i,i) edge with the same w = rs[dst] form, so fold it
    # into the edge stream (gather row i of the gcn half, scaled by rs[i])
    allnodes = np.arange(n, dtype=np.int64)
    srca_x = np.concatenate([srca, allnodes])
    dsta_x = np.concatenate([dsta, allnodes])
    gcn_streams, es_g = _edge_streams(srca_x, dsta_x, rs[dsta_x], n, shard, nt,
                                      GW * P, perm, bnd)

    f16 = np.float16

    # packed fp16 weights [P, 16*128]
    def w2(a):  # [d,128] -> list of [128,128] tiles
        a = np.asarray(a, np.float32)
        return [a[i * P:(i + 1) * P] for i in range(a.shape[0] // P)]

    wb_tiles = []
    wb_off = {}

    def put_b(name, tiles):
        wb_off[name] = len(wb_tiles) * P
        wb_tiles.extend(tiles)

    put_b("fc1", w2(inp["fc1_W"]))
    for l in (1, 2, 3):
        put_b(f"sWl{l}", w2(inp[f"s{l}_Wl"]))
        put_b(f"gW{l}", w2(inp[f"g{l}_W"]))
        put_b(f"sWr{l}", w2(inp[f"s{l}_Wr"]))
    wb = np.concatenate(wb_tiles, axis=1).astype(f16)  # [128, 16*128]

    # packed fp32 consts [P, ncols]
    wf_cols = []
    wf_off = {}

    def put_f(name, cols):
        a = np.asarray(cols, np.float32).reshape(-1)
        wf_off[name] = len(wf_cols)
        for i in range(a.shape[0] // P):
            wf_cols.append(a[i * P:(i + 1) * P])

    put_f("fc1_b", inp["fc1_b"])
    for l in (1, 2, 3):
        put_f(f"s_bl{l}", inp[f"s{l}_bl"])
        put_f(f"g_b{l}", inp[f"g{l}_b"])
    wf = np.stack(wf_cols, axis=1).astype(np.float32)  # [128, ncols]

    # fp16 head columns (scaled by w_i): [h1 | h2a h2b | h3a h3b | h4a h4b]
    w_scal = [float(inp[f"w{i}"][0]) for i in range(1, 5)]
    wh_cols = [inp["l1_W"].reshape(-1) * w_scal[0]]
    for i in (2, 3, 4):
        hw_ = inp[f"l{i}_W"].reshape(-1) * w_scal[i - 1]
        wh_cols.append(hw_[:P])
        wh_cols.append(hw_[P:])
    wh = np.stack(wh_cols, axis=1).astype(f16)  # [128, 7]
    total_bias = float(sum(float(inp[f"l{i}_b"][0]) * w_scal[i - 1]
                           for i in range(1, 5)))

    # wide iota for whole-group one-hot builds: [128, ktmx, GW*128],
    # iota[p, k, d] = d (chunk axis k broadcast on host, partitions equal)
    ktmx = max(max(es_s["Ktot"]), max(es_g["Ktot"]))
    iota = np.broadcast_to(np.arange(GW * P, dtype=np.float32),
                           (P, ktmx, GW * P)).astype(f16)
    iota = np.ascontiguousarray(iota)

    meta = dict(n=n, shard=shard, nt=nt, ktmx=ktmx, q0=q0,
                bnd=[int(b) for b in bnd],
                es_s=es_s, es_g=es_g,
                wb_off=wb_off, wf_off=wf_off, wf_cols=wf.shape[1],
                total_bias=total_bias)

    in_maps = []
    for c in range(NCORES):
        lo = c * shard
        nown = min(shard, n - lo)
        ownx = np.zeros((nt * P, P), np.float32)
        ownx[:nown] = x[lo:lo + nown]
        # feature-major own x: xto[f, t*128+j] = x_own[t*128+j, f]
        xto = np.ascontiguousarray(
            ownx.reshape(nt, P, P).transpose(2, 0, 1).reshape(P, nt * P)
        ).astype(f16)
        rso = np.ones(nt * P, np.float32)
        rso[:nown] = rs[lo:lo + nown]
        idx_s, dw_s = sage_streams[c]
        idx_g, dw_g = gcn_streams[c]
        in_maps.append({
            "xto": xto,
            "idxs": idx_s, "dws": dw_s,
            "idxg": idx_g, "dwg": dw_g,
            "wb": wb, "wf": wf, "wh": wh, "iota": iota,
            "ident": np.eye(P, dtype=f16),
            "rso": rso.reshape(nt, P).T.copy(),
        })
    return meta, in_maps


# ----------------------------------------------------------------------------
# device program
# ----------------------------------------------------------------------------

def _build(meta):
    import concourse.bacc as bacc
    import concourse.mybir as mybir
    import concourse.tile as tile

    dt = mybir.dt
    Alu = mybir.AluOpType
    Act = mybir.ActivationFunctionType

    n, shard, nt = (meta[k] for k in ("n", "shard", "nt"))
    es_s, es_g = meta["es_s"], meta["es_g"]
    SKs, SKg = es_s["SK"], es_g["SK"]
    SICs, SICg = es_s["SIC"], es_g["SIC"]
    wbo, wfo = meta["wb_off"], meta["wf_off"]

    use_barrier = bool(int(os.environ.get("GNN_BARRIER", "0")))
    AGMID = bool(int(os.environ.get("GNN_AGMID", "1")))  # emit AG r0 mid-layer
    QDED = bool(int(os.environ.get("GNN_QDED", "1")))    # region-dedicated queues
    POOL_M = int(os.environ.get("GNN_POOL_M", "0"))  # of 8 m-builds on Pool
    PROBE = os.environ.get("GNN_PROBE", "")  # timing-only, comma-separated:
    PROBE = set(p for p in PROBE.split(",") if p)  # nom|nogather|nomm|nocoll
    GP_BUFS = int(os.environ.get("GNN_GP", "4"))  # gather pool depth; startup memsets below must cover ALL bufs
    SPKT = bool(int(os.environ.get("GNN_SPKT", "1")))

    scratch = int(os.environ.get("GNN_SCRATCH", "16384"))
    NQ = int(os.environ.get("GNN_NQ", "4"))
    nc = bacc.Bacc("TRN2", target_bir_lowering=False, debug=False,
                   num_devices=NCORES, dynamic_dma_scratch_size=scratch,
                   num_swdge_queues=NQ)

    def din(name, shape, dtype):
        return nc.dram_tensor(name, shape, dtype, kind="ExternalInput")

    xto_d = din("xto", [P, nt * P], dt.float16)
    idxs_d = din("idxs", [P, SICs], dt.int16)
    dwdt = dt.float16
    dws_d = din("dws", [P, 2 * SKs], dwdt)
    idxg_d = din("idxg", [P, SICg], dt.int16)
    dwg_d = din("dwg", [P, 2 * SKg], dwdt)
    wb_d = din("wb", [P, 16 * P], dt.float16)
    wf_d = din("wf", [P, meta["wf_cols"]], dt.float32)
    wh_d = din("wh", [P, 7], dt.float16)
    ktmx = meta["ktmx"]
    iota_d = din("iota", [P, ktmx, GW * P], dt.float16)
    ident_d = din("ident", [P, P], dt.float16)
    rso_d = din("rso", [P, nt], dt.float32)
    res_d = nc.dram_tensor("res", [P, nt], dt.float32, kind="ExternalOutput")

    with tile.TileContext(nc) as tc:
        import contextlib
        _stack = contextlib.ExitStack()
        _ppool = _stack.enter_context(tc.tile_pool(name="persist", bufs=1))
        _dpool = _stack.enter_context(
            tc.tile_pool(name="persistd", bufs=1, space="DRAM"))

        def tc_tile(shape, dtype, space="SBUF", addr_space="Local", name="t"):
            pool = _dpool if space == "DRAM" else _ppool
            return pool.tile(shape, dtype, tag=name, name=name,
                             addr_space=addr_space)

        f32, f16 = dt.float32, dt.float16
        # --- persistent SBUF ---
        x0T = tc_tile([P, nt * P], f16, name="x0T")
        x1aT = tc_tile([P, nt * P], f16, name="x1aT")
        x1bT = tc_tile([P, nt * P], f16, name="x1bT")
        ngr = (nt + GW - 1) // GW
        linrg = [tc_tile([P, min(GW, nt - GW * gi) * P], f16,
                         name=f"linrg{gi}") for gi in range(ngr)]
        resb = tc_tile([P, nt], f32, name="resb")
        # per-layer head columns, evacuated by ACT; summed once at the end
        hst = tc_tile([P, 4 * nt], f16, name="hst")
        wb_s = tc_tile([P, 16 * P], f16, name="wb_s")
        wf_s = tc_tile([P, meta["wf_cols"]], f32, name="wf_s")
        wh_s = tc_tile([P, 7], f16, name="wh_s")
        iota_s = tc_tile([P, ktmx, GW * P], f16, name="iota_s")
        ident_s = tc_tile([P, P], f16, name="ident_s")
        rso_s = tc_tile([P, nt], f32, name="rso_s")
        dws_s = tc_tile([P, 2 * SKs], dwdt, name="dws_s")
        dwg_s = tc_tile([P, 2 * SKg], dwdt, name="dwg_s")
        idxs_s = tc_tile([P, SICs], dt.int16, name="idxs_s")
        idxg_s = tc_tile([P, SICg], dt.int16, name="idxg_s")

        # --- DRAM tables (split per AllGather region) ---
        q0 = meta["q0"]
        ntr0 = q0 // P
        bnd = meta["bnd"]
        rsz = [bnd[1] - bnd[0], bnd[2] - bnd[1]]
        sh = [[tc_tile([q0, 2 * P], f16, space="DRAM", name=f"sh{l}r0"),
               tc_tile([shard - q0, 2 * P], f16, space="DRAM",
                       name=f"sh{l}r1")] for l in (1, 2, 3)]
        tbl = [[tc_tile([rsz[0], 2 * P], f16, space="DRAM",
                        addr_space="Shared", name=f"tbl{l}r0"),
                tc_tile([rsz[1], 2 * P], f16, space="DRAM",
                        addr_space="Shared", name=f"tbl{l}r1")]
               for l in (1, 2, 3)]

        for t_, d_ in ((wb_s, wb_d), (wf_s, wf_d), (wh_s, wh_d),
                       (iota_s, iota_d), (ident_s, ident_d), (rso_s, rso_d),
                       (dws_s, dws_d), (dwg_s, dwg_d),
                       (idxs_s, idxs_d), (idxg_s, idxg_d)):
            nc.sync.dma_start(out=t_[:], in_=d_[:])

        KTM = {"s": max(es_s["Ktot"]), "g": max(es_g["Ktot"])}
        MP_BUFS = int(os.environ.get("GNN_MP", "3"))
        with (
            tc.tile_pool(name="xp", bufs=3) as xp,
            tc.tile_pool(name="gp", bufs=GP_BUFS) as gp,
            tc.tile_pool(name="mp", bufs=MP_BUFS) as mp,
            tc.tile_pool(name="op", bufs=4) as op,
            tc.tile_pool(name="bp", bufs=3) as bp,
            tc.tile_pool(name="pp", bufs=1, space="PSUM") as pp,
            tc.tile_pool(name="pq", bufs=3, space="PSUM") as pq,
            tc.tile_pool(name="pr", bufs=2, space="PSUM") as pr,
        ):
            def wbt(name, half=0):  # weight tile [128,128]
                o = wbo[name] + half * P
                return wb_s[:, o:o + P]

            def wfc(name, half=0):  # const col [128,1]
                o = wfo[name] + half
                return wf_s[:, o:o + 1]

            # group boundaries for batched sh writes: full groups of GW
            # tiles, forced breaks at the region boundary (ntr0) and tail
            groups = []
            t0 = 0
            while t0 < nt:
                gtiles = min(GW, nt - t0)
                if t0 < ntr0 < t0 + gtiles:     # don't straddle sh0/sh1
                    gtiles = ntr0 - t0
                if (t0 + gtiles) * P > shard:   # tail tile handled alone
                    gtiles = 1 if gtiles == 1 else gtiles - 1
                groups.append((t0, gtiles))
                t0 += gtiles

            batch_sh = bool(int(os.environ.get("GNN_BATCH_SH", "1")))

            def write_sh(sh_l, t0, gtiles, tb4):
                # route to the region tensor (groups never straddle ntr0)
                sh_t = sh_l[0] if t0 < ntr0 else sh_l[1]
                b0 = t0 * P - (0 if t0 < ntr0 else q0)
                rows = min(shard - t0 * P, gtiles * P)
                if rows == gtiles * P and batch_sh:
                    o = sh_t[b0:b0 + rows, :] \
                        .rearrange("(g p) c -> p g c", p=P)
                    i = tb4[:, :gtiles * 2 * P] \
                        .rearrange("p (g c) -> p g c", c=2 * P)
                    nc.sync.dma_start(out=o, in_=i)
                else:
                    for g in range(gtiles):
                        rt = min(P, shard - (t0 + g) * P)
                        nc.sync.dma_start(
                            out=sh_t[b0 + g * P:b0 + g * P + rt, :],
                            in_=tb4[:rt, g * 2 * P:(g + 1) * 2 * P])

            def emit_ag(l, r):
                # chunked AllGather: collects region r (sh rows [0,q0) /
                # [q0,shard) of every core) into the chunk-major tbl tensor
                if "nocoll" in PROBE:
                    return
                nc.gpsimd.collective_compute(
                    "AllGather", mybir.AluOpType.bypass,
                    replica_groups=[list(range(NCORES))],
                    ins=[sh[l][r][:]], outs=[tbl[l][r][:]])

            # init the gather pool buffers once: trimmed gather calls leave
            # pad slots unwritten, and first-use SBUF could decode as NaN
            # (NaN * zero-m = NaN in the matmul)
            for kind_, es_ in (("s", es_s), ("g", es_g)):
                ktm = max(es_["Ktot"])
                for _ in range(GP_BUFS):
                    gz = gp.tile([P, ktm * P], f16, tag="gath" + kind_)
                    nc.vector.memset(gz[:], 0.0)

            # ---- layer 1 prologue: own-shard table + x0/linr/head ----
            for (t0, gtiles) in groups:
                xo4 = xp.tile([P, 4 * P], f16, tag="xo4")
                nc.sync.dma_start(out=xo4[:, :gtiles * P],
                                  in_=xto_d[:, t0 * P:(t0 + gtiles) * P])
                tb4 = bp.tile([P, 4 * 2 * P], f16, tag="tb4")
                for g in range(gtiles):
                    t = t0 + g
                    sl = slice(t * P, (t + 1) * P)
                    p1 = pq.tile([P, P], f32, tag="pa")
                    nc.tensor.matmul(p1[:], lhsT=wbt("fc1"),
                                     rhs=xo4[:, g * P:(g + 1) * P],
                                     start=True, stop=True)
                    nc.scalar.activation(x0T[:, sl], p1[:], Act.Relu,
                                         bias=wfc("fc1_b"))
                    ps = pp.tile([P, P], f32, tag="tbs")
                    nc.tensor.matmul(ps[:], lhsT=x0T[:, sl], rhs=wbt("sWl1"),
                                     start=True, stop=True)
                    pg = pp.tile([P, P], f32, tag="tbg")
                    nc.tensor.matmul(pg[:], lhsT=x0T[:, sl], rhs=wbt("gW1"),
                                     start=True, stop=True)
                    nc.scalar.activation(tb4[:, g * 2 * P:g * 2 * P + P],
                                         ps[:], Act.Copy)
                    nc.scalar.activation(tb4[:, g * 2 * P + P:(g + 1) * 2 * P],
                                         pg[:], Act.Copy,
                                         scale=rso_s[:, t:t + 1])
                write_sh(sh[0], t0, gtiles, tb4)
                if t0 + gtiles == ntr0 and AGMID:
                    emit_ag(0, 0)

            if use_barrier:
                tc.strict_bb_all_engine_barrier()
            if not AGMID:
                emit_ag(0, 0)
            emit_ag(0, 1)
            # linr/head work is independent of the collective — issued after
            # it so it executes during the AllGather instead of gating it
            for t in range(nt):
                sl = slice(t * P, (t + 1) * P)
                plr = pp.tile([P, GW * P], f32, tag="plr")
                nc.tensor.matmul(plr[:, :P], lhsT=wbt("sWr1"), rhs=x0T[:, sl],
                                 start=True, stop=False)
                nc.tensor.matmul(plr[:, :P], lhsT=ident_s[:], rhs=x0T[:, sl],
                                 start=False, stop=True)
                nc.scalar.activation(
                    linrg[t // GW][:, (t % GW) * P:(t % GW + 1) * P],
                    plr[:, :P], Act.Identity, bias=wfc("s_bl1"))
                ph = pr.tile([P, 1], f32, tag="ph")
                nc.tensor.matmul(ph[:], lhsT=x0T[:, sl],
                                 rhs=wh_s[:, 0:1], start=True, stop=True)
                nc.scalar.activation(hst[:, t:t + 1], ph[:], Act.Copy)
            if use_barrier:
                tc.strict_bb_all_engine_barrier()

            # ---- conv layers ----
            qctr = [0]

            def conv_group(kind, gi, W, tbl_t):
                """Aggregate one dst-group (W = gtiles*128 nodes) of one edge
                set into a [128, W] psum via per-region gathers + 512-wide
                one-hot matmuls."""
                es = es_s if kind == "s" else es_g
                idx_sb = idxs_s if kind == "s" else idxg_s
                dw = dws_s if kind == "s" else dwg_s
                SK = es["SK"]
                nreg = es["nreg"]
                colofs = 0 if kind == "s" else P
                pa = pq.tile([P, GW * P], f32, tag="pa")
                Kt = es["Ktot"][gi]
                off = es["coff"][gi][0]
                g = gp.tile([P, Kt * P], f16, tag="gath" + kind)
                if "nogather" in PROBE:
                    nc.vector.memset(g[:], 0.125)
                else:
                    for r in range(nreg):
                        kr = es["K"][gi][r]
                        cv = es["CV"][gi][r]
                        io = es["ioff"][gi][r]
                        co = es["coff"][gi][r] - off
                        for s in range(0, kr, CAPK):
                            kk = min(CAPK, kr - s)
                            vcall = min(kk * P, cv - s * P)
                            if vcall <= 0:
                                continue  # all-pad tail: zero-m, never read
                            qctr[0] += 1
                            # region r rides queues {2r, 2r+1}: region-1
                            # calls waiting on their AllGather never block
                            # region-0 calls (swdge queues are FIFO)
                            nc.gpsimd.dma_gather(
                                out_ap=g[:, (co + s) * P:(co + s + kk) * P]
                                .rearrange("p (k e) -> p k e", e=P),
                                in_ap=tbl_t[r][:, colofs:colofs + P],
                                idxs_ap=idx_sb[:, io + s * 8:io + (s + kk) * 8],
                                num_idxs=vcall,
                                num_idxs_reg=vcall,
                                elem_size=P,
                                elem_step=2 * P,
                                single_packet=SPKT,
                                queue_num=((2 * r + qctr[0] % 2) % NQ
                                           if QDED else qctr[0] % NQ))
                # whole-group one-hot build: 2 wide DVE ops; dl/wv columns are
                # stride-0 broadcast along the dst axis (layer-independent)
                m = mp.tile([P, KTM[kind], P], f16, tag="m" + kind)
                if "nom" in PROBE:
                    nc.vector.memset(m[:, :Kt, :W], 0.125)
                else:
                    dlb = dw[:, off:off + Kt].broadcast_to([P, Kt, W])
                    wvb = dw[:, SK + off:SK + off + Kt] \
                        .broadcast_to([P, Kt, W])
                    nc.vector.tensor_tensor(out=m[:, :Kt, :W],
                                            in0=iota_s[:, :Kt, :W],
                                            in1=dlb, op=Alu.is_equal)
                    nc.vector.tensor_tensor(out=m[:, :Kt, :W],
                                            in0=m[:, :Kt, :W],
                                            in1=wvb, op=Alu.mult)
                # accumulation group stays OPEN: the caller closes it with
                # identity-matmul residual adds (PE-side fused finalize)
                for k in range(Kt):
                    if "nomm" in PROBE and k > 0:
                        continue
                    nc.tensor.matmul(pa[:, :W], lhsT=g[:, k * P:(k + 1) * P],
                                     rhs=m[:, k, :W], start=(k == 0),
                                     stop=False)
                return pa

            for l in (1, 2, 3):
                tbl_t = [tbl[l - 1][0][:], tbl[l - 1][1][:]]
                sh_next = sh[l] if l < 3 else None
                for gi, (t0, gtiles) in enumerate(groups):
                    W = gtiles * P
                    gsl = slice(t0 * P, t0 * P + W)
                    if l < 3:
                        tb4 = bp.tile([P, 4 * 2 * P], f16, tag="tb4")
                    else:
                        tb4 = None
                    # both conv chains BEFORE either finalize: the finalize
                    # waits on its psum (gathers+matmuls), and the in-order
                    # DVE queue would otherwise stall the second conv's
                    # one-hot builds behind it
                    # residual adds ride the PE as identity matmuls appended
                    # to each conv's accumulation group (groups stay
                    # contiguous per psum tile); evacuations ride ACT with
                    # fused bias. DVE only does head accumulation here.
                    pa = conv_group("s", gi, W, tbl_t)
                    # ocf = agg + linr (linr includes bl + x0 (+x1a))
                    nc.tensor.matmul(pa[:, :W], lhsT=ident_s[:],
                                     rhs=linrg[gi][:], start=False, stop=True)
                    pg = conv_group("g", gi, W, tbl_t)
                    # oaf = agg + x0 (+x1b) + g_b
                    nc.tensor.matmul(pg[:, :W], lhsT=ident_s[:],
                                     rhs=x0T[:, gsl], start=False,
                                     stop=(l == 1))
                    if l > 1:
                        nc.tensor.matmul(pg[:, :W], lhsT=ident_s[:],
                                         rhs=x1bT[:, gsl], start=False,
                                         stop=True)
                    if l == 1:
                        ocf = x1aT[:, gsl]
                        oaf = x1bT[:, gsl]
                    else:
                        ocf_t = op.tile([P, 4 * P], f16, tag="ocf")
                        ocf = ocf_t[:, :W]
                        oaf_t = op.tile([P, 4 * P], f16, tag="oaf")
                        oaf = oaf_t[:, :W]
                    nc.scalar.activation(ocf, pa[:, :W], Act.Copy)
                    nc.scalar.activation(oaf, pg[:, :W], Act.Identity,
                                         bias=wfc(f"g_b{l}"))
                    # heads on out_{l+1} (per tile: lhsT free dim <= 128)
                    hc = 1 + 2 * (l - 1)
                    for j in range(gtiles):
                        t = t0 + j
                        jsl = slice(j * P, (j + 1) * P)
                        ph = pr.tile([P, 1], f32, tag="ph")
                        nc.tensor.matmul(ph[:], lhsT=ocf[:, jsl],
                                         rhs=wh_s[:, hc:hc + 1],
                                         start=True, stop=False)
                        nc.tensor.matmul(ph[:], lhsT=oaf[:, jsl],
                                         rhs=wh_s[:, hc + 1:hc + 2],
                                         start=False, stop=True)
                        nc.scalar.activation(hst[:, l * nt + t:l * nt + t + 1],
                                             ph[:], Act.Copy)
                    if l == 3:
                        continue
                    # ---- boundary: tables (per tile) + linr (per group) ----
                    ln = l + 1
                    for j in range(gtiles):
                        t = t0 + j
                        jsl = slice(j * P, (j + 1) * P)
                        ps = pp.tile([P, P], f32, tag="tbs")
                        nc.tensor.matmul(ps[:], lhsT=ocf[:, jsl],
                                         rhs=wbt(f"sWl{ln}", 0),
                                         start=True, stop=False)
                        nc.tensor.matmul(ps[:], lhsT=oaf[:, jsl],
                                         rhs=wbt(f"sWl{ln}", 1),
                                         start=False, stop=True)
                        pgt = pp.tile([P, P], f32, tag="tbg")
                        nc.tensor.matmul(pgt[:], lhsT=ocf[:, jsl],
                                         rhs=wbt(f"gW{ln}", 0),
                                         start=True, stop=False)
                        nc.tensor.matmul(pgt[:], lhsT=oaf[:, jsl],
                                         rhs=wbt(f"gW{ln}", 1),
                                         start=False, stop=True)
                        nc.scalar.activation(tb4[:, j * 2 * P:j * 2 * P + P],
                                             ps[:], Act.Copy)
                        nc.scalar.activation(
                            tb4[:, j * 2 * P + P:(j + 1) * 2 * P],
                            pgt[:], Act.Copy, scale=rso_s[:, t:t + 1])
                    plr = pp.tile([P, GW * P], f32, tag="plr")
                    nc.tensor.matmul(plr[:, :W], lhsT=wbt(f"sWr{ln}", 0),
                                     rhs=ocf, start=True, stop=False)
                    nc.tensor.matmul(plr[:, :W], lhsT=wbt(f"sWr{ln}", 1),
                                     rhs=oaf, start=False, stop=False)
                    nc.tensor.matmul(plr[:, :W], lhsT=ident_s[:],
                                     rhs=x0T[:, gsl], start=False, stop=False)
                    nc.tensor.matmul(plr[:, :W], lhsT=ident_s[:],
                                     rhs=x1aT[:, gsl], start=False, stop=True)
                    nc.scalar.activation(linrg[gi][:], plr[:, :W],
                                         Act.Identity, bias=wfc(f"s_bl{ln}"))
                    write_sh(sh_next, t0, gtiles, tb4)
                    if t0 + gtiles == ntr0 and AGMID:
                        emit_ag(l, 0)
                if l < 3:
                    if use_barrier:
                        tc.strict_bb_all_engine_barrier()
                    if not AGMID:
                        emit_ag(l, 0)
                    emit_ag(l, 1)
                    if use_barrier:
                        tc.strict_bb_all_engine_barrier()

            # ---- output ----
            nc.vector.tensor_tensor(out=resb[:], in0=hst[:, 0:nt],
                                    in1=hst[:, nt:2 * nt], op=Alu.add)
            nc.vector.tensor_tensor(out=resb[:], in0=resb[:],
                                    in1=hst[:, 2 * nt:3 * nt], op=Alu.add)
            nc.vector.scalar_tensor_tensor(
                out=resb[:], in0=hst[:, 3 * nt:4 * nt],
                scalar=float(meta["total_bias"]), in1=resb[:],
                op0=Alu.add, op1=Alu.add)
            nc.sync.dma_start(out=res_d[:], in_=resb[:])
        _stack.close()

    nc.compile()
    return nc


# ----------------------------------------------------------------------------
# entry point
# ----------------------------------------------------------------------------

def _run_and_bench(nc, in_maps, iters):
    """Mirror bass2jax.run_bass_via_pjrt's multi-core path, plus an optional
    pipelined repeat loop to measure marginal per-execution device time."""
    import time
    import jax
    import numpy as np
    from jax.sharding import Mesh, PartitionSpec
    from jax.experimental.shard_map import shard_map
    import concourse.mybir as mybir
    from concourse import bass2jax

    bass2jax.install_neuronx_cc_hook()
    partition_name = (nc.partition_id_tensor.name
                      if nc.partition_id_tensor else None)
    in_names, out_names, out_avals, zero_outs = [], [], [], []
    for alloc in nc.m.functions[0].allocations:
        if not isinstance(alloc, mybir.MemoryLocationSet):
            continue
        name = alloc.memorylocations[0].name
        if alloc.kind == "ExternalInput":
            if name != partition_name:
                in_names.append(name)
        elif alloc.kind == "ExternalOutput":
            shape = tuple(alloc.tensor_shape)
            dtype = mybir.dt.np(alloc.dtype)
            out_names.append(name)
            out_avals.append(jax.core.ShapedArray(shape, dtype))
            zero_outs.append(np.zeros(shape, dtype))
    n_params = len(in_names)
    all_in_names = list(in_names) + out_names
    if partition_name is not None:
        all_in_names.append(partition_name)

    def _body(*args):
        operands = list(args)
        if partition_name is not None:
            operands.append(bass2jax.partition_id_tensor())
        outs = bass2jax._bass_exec_p.bind(
            *operands, out_avals=tuple(out_avals),
            in_names=tuple(all_in_names), out_names=tuple(out_names),
            lowering_input_output_aliases=(),
            sim_require_finite=True, sim_require_nnan=True, nc=nc)
        return tuple(outs)

    devices = jax.devices()[:NCORES]
    mesh = Mesh(np.asarray(devices), ("core",))
    in_specs = (PartitionSpec("core"),) * (n_params + len(out_names))
    out_specs = (PartitionSpec("core"),) * len(out_names)
    sharded = jax.jit(shard_map(_body, mesh=mesh, in_specs=in_specs,
                                out_specs=out_specs, check_rep=False),
                      keep_unused=True)
    concat_in = [
        np.concatenate([np.asarray(in_maps[c][nm]) for c in range(NCORES)], 0)
        for nm in in_names]
    concat_zeros = [np.zeros((NCORES * z.shape[0], *z.shape[1:]), z.dtype)
                    for z in zero_outs]
    out_arrs = sharded(*concat_in, *concat_zeros)
    jax.block_until_ready(out_arrs)

    per_exec_ns = None
    if iters > 0:
        from jax.sharding import NamedSharding
        dev_in = [jax.device_put(a, NamedSharding(mesh, PartitionSpec("core")))
                  for a in concat_in]
        dev_zero = [jax.device_put(z, NamedSharding(mesh, PartitionSpec("core")))
                    for z in concat_zeros]
        r = sharded(*dev_in, *dev_zero)
        jax.block_until_ready(r)
        batches = int(os.environ.get("GNN_BATCHES", "10"))
        best = None
        for _ in range(batches):
            t1 = time.perf_counter()
            rs = [sharded(*dev_in, *dev_zero) for _ in range(iters)]
            jax.block_until_ready(rs)
            t2 = time.perf_counter()
            cur = (t2 - t1) / iters * 1e9
            best = cur if best is None else min(best, cur)
        per_exec_ns = best

    results = [
        {nm: np.asarray(out_arrs[i]).reshape(NCORES, *out_avals[i].shape)[c]
         for i, nm in enumerate(out_names)}
        for c in range(NCORES)]
    return results, per_exec_ns


def kernel(**inputs):
    global LAST_EXEC_NS, LAST_TRACE

    meta, in_maps = _prep(inputs)
    nc = _build(meta)

    iters = int(os.environ.get("GNN_BENCH", "0"))
    results, per_exec_ns = _run_and_bench(nc, in_maps, iters)
    LAST_EXEC_NS = per_exec_ns
    LAST_TRACE = None

    n, shard, nt = meta["n"], meta["shard"], meta["nt"]
    out = np.empty((n, 1), np.float32)
    for c in range(NCORES):
        r = results[c]["res"]  # [128, nt]
        out[c * shard:(c + 1) * shard, 0] = r.T.reshape(-1)[:shard]
    return out



# revision 45
# speedup vs baseline: 1.0950x; 1.0950x over previous
"""Trainium2 Bass kernel for a 3-layer SAGE+GCN GNN on 50k nodes / 800k edges,
sharded across 8 NeuronCores.

Strategy (v2):
  - Nodes sharded into 8 contiguous ranges; edges assigned to the core that
    owns their dst node (host sorts edges by dst tile / src region).
  - Per conv, inputs are pre-projected to 128-dim fp16 "table" rows
    ([node, 256] = sage half | gcn half); every layer's table is built from
    the core's own shard and AllGathered (layer 1 included — no replicated
    full-graph projection).
  - Aggregation: indirect-DMA gather of each dst-tile's edge rows into SBUF,
    then one-hot matmul segment-sum on TensorE (M[e,d] = (dst[e]==d) * w_e
    built on VectorE; w_e folds the SAGE 1/deg or GCN norm).
  - idx16 gather indices are persistent in SBUF (loaded once), removing
    per-gather DMA dispatches.
  - Everything node-indexed on-chip is feature-major ([feat, node]); psum
    evacuations ride the Activation engine; residual adds on VectorE in fp16.

Measured-on-HW facts that shaped this (timing probes, 2026-08):
  - dma_gather is ~100% of device time; spreading gathers round-robin over
    num_swdge_queues=4 (queue_num=) gave ~3x on the gather phase. One queue
    runs ~60 GB/s effective; 4 queues ~119 GB/s (close to the HBM
    random-256B-row wall).
  - A single gather call is capped by the 1024-entry descriptor ring
    (CAPK=8 chunks x 128 rows works; larger crashes NRT).
  - Pool/GPSIMD ALU ops cost ~3 us each on HW — never offload elementwise
    work there (POOL_M stays 0).
  - Real AllGather (3.2MB in / 25.6MB out, 8 cores) is ~70-90 us, 4x faster
    than the cost model claims; barriers around collectives are unnecessary
    (Tile deps suffice) and removed.
  - fp16 tables/weights (vs bf16) cost nothing and cut rel err ~5x.
  - num_idxs need not be a multiple of 128: trimming each cell's final
    gather call to the true 16-rounded row count drops the ~12% pad
    descriptors (requires the startup memsets of the gather pool buffers —
    unwritten slots are zero-m'd but NaN*0=NaN would poison the psum).
  - In-cell src-address sorting and single_packet=0 measured neutral: the
    gather wall is descriptor-processing rate, not DRAM row locality.
"""

import os
import numpy as np

P = 128
NCORES = 8

LAST_EXEC_NS = None
LAST_TRACE = None


# ----------------------------------------------------------------------------
# host-side preprocessing
# ----------------------------------------------------------------------------

CAPK = int(os.environ.get("GNN_CAPK", "8"))    # chunks per dma_gather call
GW = int(os.environ.get("GNN_GW", "1"))        # dst tiles per one-hot group


def _edge_streams(src, dst, w_edge, n, shard, nt, tw, perm, bnd):
    """Per-core gather/M-build streams for one edge set, organized for
    nc.gpsimd.dma_gather: per (dst-group g of tw nodes, src-region r)
    gathers of NV[g][r] valid rows (idx-0 padded to a cross-core-uniform
    count). dst-local offsets (dl) are relative to the group base
    (0..tw-1, exact in fp16 for tw<=2048).

    Regions are defined in PERMUTED (chunk-major) table space via `bnd`
    boundaries so each region's table slice is produced by its own
    chunked AllGather (perm[n] = permuted row of node n).

    Returns per-core (idx16 [128, SIC], dw [128, 2*SK]) plus layout lists.
    """
    nreg = len(bnd) - 1
    ngr = (nt * P + tw - 1) // tw
    percore = []
    counts = np.zeros((NCORES, ngr, nreg), np.int64)
    for c in range(NCORES):
        lo, hi = c * shard, (c + 1) * shard
        m = (dst >= lo) & (dst < hi)
        s_c, d_c, w_c = perm[src[m]], dst[m] - lo, w_edge[m]
        reg_c = np.searchsorted(bnd, s_c, side="right") - 1
        # by (group, region), then ascending src address within the cell —
        # the in-cell order is free (dl/wv follow it) and sorted addresses
        # give the DMA engines DRAM row/bank locality on the random reads
        order = np.lexsort((s_c, reg_c, d_c // tw))
        s_c, d_c, w_c, reg_c = s_c[order], d_c[order], w_c[order], reg_c[order]
        key = (d_c // tw) * nreg + reg_c
        bounds = np.searchsorted(key, np.arange(ngr * nreg + 1))
        counts[c] = (bounds[1:] - bounds[:-1]).reshape(ngr, nreg)
        percore.append((s_c, d_c, w_c, bounds))
    NV = counts.max(axis=0)                       # [ngr, nreg] max real count
    K = np.maximum((NV + P - 1) // P, 1)          # chunks per (g, r)
    # CV: true gather row count per cell, 16-rounded — the final gather call
    # of a cell is trimmed to this (pad slots beyond it are never fetched;
    # their m columns are zero so stale SBUF data cannot contribute)
    CV = np.minimum((NV + 15) // 16 * 16, K * P)
    NV = K * P                                    # all slots valid (idx-0 pads)
    Ktot = K.sum(axis=1)                          # chunks per group
    SK = int(Ktot.sum())
    coff = np.zeros((ngr, nreg), np.int64)
    ioff = np.zeros((ngr, nreg), np.int64)
    acc_c = 0
    acc_i = 0
    for t in range(ngr):
        for r in range(nreg):
            coff[t, r] = acc_c
            ioff[t, r] = acc_i
            acc_c += K[t, r]
            acc_i += K[t, r] * 8
    SIC = int(acc_i)

    outs = []
    for c in range(NCORES):
        s_c, d_c, w_c, bounds = percore[c]
        idx16 = np.zeros((P, SIC), np.int16)
        dl = np.full((P, SK), -1.0, np.float32)
        wv = np.zeros((P, SK), np.float32)
        for t in range(ngr):
            for r in range(nreg):
                b0, b1 = bounds[t * nreg + r], bounds[t * nreg + r + 1]
                cnt = b1 - b0
                kr = int(K[t, r])
                slots = kr * P
                buf_i = np.zeros(slots, np.int32)   # idx-0 pads (always write)
                buf_i[:cnt] = s_c[b0:b1] - bnd[r]
                buf_d = np.full(slots, -1.0, np.float32)
                buf_d[:cnt] = (d_c[b0:b1] - t * tw).astype(np.float32)
                buf_w = np.zeros(slots, np.float32)
                buf_w[:cnt] = w_c[b0:b1]
                # idx16: wrapped over 16 partitions, replicated x8
                cols = kr * 8
                wrap = buf_i.reshape(cols, 16).T.astype(np.int16)  # [16, cols]
                io = int(ioff[t, r])
                idx16[:, io:io + cols] = np.tile(wrap, (8, 1))
                # dl/wv: slot i -> partition i%128, chunk coff+i//128
                co = int(coff[t, r])
                dl[:, co:co + kr] = buf_d.reshape(kr, P).T
                wv[:, co:co + kr] = buf_w.reshape(kr, P).T
        # fp16: dl values 0..tw-1 are exact; wv rounding is ~5e-4 rel
        dw = np.concatenate([dl, wv], axis=1).astype(np.float16)
        outs.append((idx16, dw))
    meta_es = dict(K=K.tolist(), NV=NV.tolist(), CV=CV.tolist(),
                   Ktot=[int(x) for x in Ktot],
                   coff=coff.tolist(), ioff=ioff.tolist(),
                   SK=SK, SIC=SIC, nreg=nreg, ngr=ngr)
    return outs, meta_es


def _prep(inputs):
    inp = {k: np.asarray(v) for k, v in inputs.items()}
    x = inp["x"].astype(np.float32)
    n, din = x.shape
    assert din == P
    shard = n // NCORES
    nt = (shard + P - 1) // P

    src = inp["edge_index"][0].astype(np.int64)
    dst = inp["edge_index"][1].astype(np.int64)
    srca = inp["edge_index_aux"][0].astype(np.int64)
    dsta = inp["edge_index_aux"][1].astype(np.int64)

    deg = np.zeros(n, np.float32)
    np.add.at(deg, dst, 1.0)
    recip_deg = (1.0 / np.maximum(deg, 1.0)).astype(np.float32)
    dega = np.zeros(n, np.float32)
    np.add.at(dega, dsta, 1.0)
    deg_hat = dega + 1.0
    rs = (1.0 / np.sqrt(deg_hat)).astype(np.float32)

    # regions are int16 idx windows into the (unpermuted) table; a split
    # chunk-major AllGather was tried and measured WORSE (each collective
    # carries a ~180us fixed/barrier cost, so 6 AGs lose to 3)
    nregions = int(os.environ.get("GNN_NREG", "2"))
    bnd = np.linspace(0, n, nregions + 1).astype(np.int64)
    perm = np.arange(n, dtype=np.int64)
    assert int(np.diff(bnd).max()) < 32768  # int16 gather offsets

    sage_streams, es_s = _edge_streams(src, dst, recip_deg[dst], n, shard, nt,
                                       GW * P, perm, bnd)
    # GCN self-loop == a (i,i) edge with the same w = rs[dst] form, so fold it
    # into the edge stream (gather row i of the gcn half, scaled by rs[i])
    allnodes = np.arange(n, dtype=np.int64)
    srca_x = np.concatenate([srca, allnodes])
    dsta_x = np.concatenate([dsta, allnodes])
    gcn_streams, es_g = _edge_streams(srca_x, dsta_x, rs[dsta_x], n, shard, nt,
                                      GW * P, perm, bnd)

    f16 = np.float16

    # packed fp16 weights [P, 16*128]
    def w2(a):  # [d,128] -> list of [128,128] tiles
        a = np.asarray(a, np.float32)
        return [a[i * P:(i + 1) * P] for i in range(a.shape[0] // P)]

    wb_tiles = []
    wb_off = {}

    def put_b(name, tiles):
        wb_off[name] = len(wb_tiles) * P
        wb_tiles.extend(tiles)

    put_b("fc1", w2(inp["fc1_W"]))
    for l in (1, 2, 3):
        put_b(f"sWl{l}", w2(inp[f"s{l}_Wl"]))
        put_b(f"gW{l}", w2(inp[f"g{l}_W"]))
        put_b(f"sWr{l}", w2(inp[f"s{l}_Wr"]))
    wb = np.concatenate(wb_tiles, axis=1).astype(f16)  # [128, 16*128]

    # packed fp32 consts [P, ncols]
    wf_cols = []
    wf_off = {}

    def put_f(name, cols):
        a = np.asarray(cols, np.float32).reshape(-1)
        wf_off[name] = len(wf_cols)
        for i in range(a.shape[0] // P):
            wf_cols.append(a[i * P:(i + 1) * P])

    put_f("fc1_b", inp["fc1_b"])
    for l in (1, 2, 3):
        put_f(f"s_bl{l}", inp[f"s{l}_bl"])
        put_f(f"g_b{l}", inp[f"g{l}_b"])
    wf = np.stack(wf_cols, axis=1).astype(np.float32)  # [128, ncols]

    # fp16 head columns (scaled by w_i): [h1 | h2a h2b | h3a h3b | h4a h4b]
    w_scal = [float(inp[f"w{i}"][0]) for i in range(1, 5)]
    wh_cols = [inp["l1_W"].reshape(-1) * w_scal[0]]
    for i in (2, 3, 4):
        hw_ = inp[f"l{i}_W"].reshape(-1) * w_scal[i - 1]
        wh_cols.append(hw_[:P])
        wh_cols.append(hw_[P:])
    wh = np.stack(wh_cols, axis=1).astype(f16)  # [128, 7]
    total_bias = float(sum(float(inp[f"l{i}_b"][0]) * w_scal[i - 1]
                           for i in range(1, 5)))

    # wide iota for whole-group one-hot builds: [128, ktmx, GW*128],
    # iota[p, k, d] = d (chunk axis k broadcast on host, partitions equal)
    ktmx = max(max(es_s["Ktot"]), max(es_g["Ktot"]))
    iota = np.broadcast_to(np.arange(GW * P, dtype=np.float32),
                           (P, ktmx, GW * P)).astype(f16)
    iota = np.ascontiguousarray(iota)

    meta = dict(n=n, shard=shard, nt=nt, ktmx=ktmx,
                bnd=[int(b) for b in bnd],
                es_s=es_s, es_g=es_g,
                wb_off=wb_off, wf_off=wf_off, wf_cols=wf.shape[1],
                total_bias=total_bias)

    in_maps = []
    for c in range(NCORES):
        lo = c * shard
        nown = min(shard, n - lo)
        ownx = np.zeros((nt * P, P), np.float32)
        ownx[:nown] = x[lo:lo + nown]
        # feature-major own x: xto[f, t*128+j] = x_own[t*128+j, f]
        xto = np.ascontiguousarray(
            ownx.reshape(nt, P, P).transpose(2, 0, 1).reshape(P, nt * P)
        ).astype(f16)
        rso = np.ones(nt * P, np.float32)
        rso[:nown] = rs[lo:lo + nown]
        idx_s, dw_s = sage_streams[c]
        idx_g, dw_g = gcn_streams[c]
        in_maps.append({
            "xto": xto,
            "idxs": idx_s, "dws": dw_s,
            "idxg": idx_g, "dwg": dw_g,
            "wb": wb, "wf": wf, "wh": wh, "iota": iota,
            "ident": np.eye(P, dtype=f16),
            "rso": rso.reshape(nt, P).T.copy(),
        })
    return meta, in_maps


# ----------------------------------------------------------------------------
# device program
# ----------------------------------------------------------------------------

def _build(meta):
    import concourse.bacc as bacc
    import concourse.mybir as mybir
    import concourse.tile as tile

    dt = mybir.dt
    Alu = mybir.AluOpType
    Act = mybir.ActivationFunctionType

    n, shard, nt = (meta[k] for k in ("n", "shard", "nt"))
    es_s, es_g = meta["es_s"], meta["es_g"]
    SKs, SKg = es_s["SK"], es_g["SK"]
    SICs, SICg = es_s["SIC"], es_g["SIC"]
    wbo, wfo = meta["wb_off"], meta["wf_off"]

    use_barrier = bool(int(os.environ.get("GNN_BARRIER", "0")))
    POOL_M = int(os.environ.get("GNN_POOL_M", "0"))  # of 8 m-builds on Pool
    PROBE = os.environ.get("GNN_PROBE", "")  # timing-only, comma-separated:
    PROBE = set(p for p in PROBE.split(",") if p)  # nom|nogather|nomm|nocoll
    GP_BUFS = int(os.environ.get("GNN_GP", "4"))  # gather pool depth; startup memsets below must cover ALL bufs
    SPKT = bool(int(os.environ.get("GNN_SPKT", "1")))

    scratch = int(os.environ.get("GNN_SCRATCH", "16384"))
    NQ = int(os.environ.get("GNN_NQ", "4"))
    nc = bacc.Bacc("TRN2", target_bir_lowering=False, debug=False,
                   num_devices=NCORES, dynamic_dma_scratch_size=scratch,
                   num_swdge_queues=NQ)

    def din(name, shape, dtype):
        return nc.dram_tensor(name, shape, dtype, kind="ExternalInput")

    xto_d = din("xto", [P, nt * P], dt.float16)
    idxs_d = din("idxs", [P, SICs], dt.int16)
    dwdt = dt.float16
    dws_d = din("dws", [P, 2 * SKs], dwdt)
    idxg_d = din("idxg", [P, SICg], dt.int16)
    dwg_d = din("dwg", [P, 2 * SKg], dwdt)
    wb_d = din("wb", [P, 16 * P], dt.float16)
    wf_d = din("wf", [P, meta["wf_cols"]], dt.float32)
    wh_d = din("wh", [P, 7], dt.float16)
    ktmx = meta["ktmx"]
    iota_d = din("iota", [P, ktmx, GW * P], dt.float16)
    ident_d = din("ident", [P, P], dt.float16)
    rso_d = din("rso", [P, nt], dt.float32)
    res_d = nc.dram_tensor("res", [P, nt], dt.float32, kind="ExternalOutput")

    with tile.TileContext(nc) as tc:
        import contextlib
        _stack = contextlib.ExitStack()
        _ppool = _stack.enter_context(tc.tile_pool(name="persist", bufs=1))
        _dpool = _stack.enter_context(
            tc.tile_pool(name="persistd", bufs=1, space="DRAM"))

        def tc_tile(shape, dtype, space="SBUF", addr_space="Local", name="t"):
            pool = _dpool if space == "DRAM" else _ppool
            return pool.tile(shape, dtype, tag=name, name=name,
                             addr_space=addr_space)

        f32, f16 = dt.float32, dt.float16
        # --- persistent SBUF ---
        x0T = tc_tile([P, nt * P], f16, name="x0T")
        x1aT = tc_tile([P, nt * P], f16, name="x1aT")
        x1bT = tc_tile([P, nt * P], f16, name="x1bT")
        ngr = (nt + GW - 1) // GW
        linrg = [tc_tile([P, min(GW, nt - GW * gi) * P], f16,
                         name=f"linrg{gi}") for gi in range(ngr)]
        resb = tc_tile([P, nt], f32, name="resb")
        # per-layer head columns, evacuated by ACT; summed once at the end
        hst = tc_tile([P, 4 * nt], f16, name="hst")
        wb_s = tc_tile([P, 16 * P], f16, name="wb_s")
        wf_s = tc_tile([P, meta["wf_cols"]], f32, name="wf_s")
        wh_s = tc_tile([P, 7], f16, name="wh_s")
        iota_s = tc_tile([P, ktmx, GW * P], f16, name="iota_s")
        ident_s = tc_tile([P, P], f16, name="ident_s")
        rso_s = tc_tile([P, nt], f32, name="rso_s")
        dws_s = tc_tile([P, 2 * SKs], dwdt, name="dws_s")
        dwg_s = tc_tile([P, 2 * SKg], dwdt, name="dwg_s")
        idxs_s = tc_tile([P, SICs], dt.int16, name="idxs_s")
        idxg_s = tc_tile([P, SICg], dt.int16, name="idxg_s")

        # --- DRAM tables ---
        bnd = meta["bnd"]
        sh = [tc_tile([shard, 2 * P], f16, space="DRAM", name=f"sh{l}")
              for l in (1, 2, 3)]
        tbl = [tc_tile([n, 2 * P], f16, space="DRAM", addr_space="Shared",
                       name=f"tbl{l}") for l in (1, 2, 3)]

        for t_, d_ in ((wb_s, wb_d), (wf_s, wf_d), (wh_s, wh_d),
                       (iota_s, iota_d), (ident_s, ident_d), (rso_s, rso_d),
                       (dws_s, dws_d), (dwg_s, dwg_d),
                       (idxs_s, idxs_d), (idxg_s, idxg_d)):
            nc.sync.dma_start(out=t_[:], in_=d_[:])

        KTM = {"s": max(es_s["Ktot"]), "g": max(es_g["Ktot"])}
        MP_BUFS = int(os.environ.get("GNN_MP", "3"))
        with (
            tc.tile_pool(name="xp", bufs=3) as xp,
            tc.tile_pool(name="gp", bufs=GP_BUFS) as gp,
            tc.tile_pool(name="mp", bufs=MP_BUFS) as mp,
            tc.tile_pool(name="op", bufs=4) as op,
            tc.tile_pool(name="bp", bufs=3) as bp,
            tc.tile_pool(name="pp", bufs=1, space="PSUM") as pp,
            tc.tile_pool(name="pq", bufs=3, space="PSUM") as pq,
            tc.tile_pool(name="pr", bufs=2, space="PSUM") as pr,
        ):
            def wbt(name, half=0):  # weight tile [128,128]
                o = wbo[name] + half * P
                return wb_s[:, o:o + P]

            def wfc(name, half=0):  # const col [128,1]
                o = wfo[name] + half
                return wf_s[:, o:o + 1]

            # group boundaries for batched sh writes: full groups of GW tiles
            groups = []
            t0 = 0
            while t0 < nt:
                gtiles = min(GW, nt - t0)
                if (t0 + gtiles) * P > shard:   # tail tile handled alone
                    gtiles = 1 if gtiles == 1 else gtiles - 1
                groups.append((t0, gtiles))
                t0 += gtiles

            batch_sh = bool(int(os.environ.get("GNN_BATCH_SH", "1")))

            def write_sh(sh_t, t0, gtiles, tb4):
                rows = min(shard - t0 * P, gtiles * P)
                if rows == gtiles * P and batch_sh:
                    o = sh_t[t0 * P:t0 * P + rows, :] \
                        .rearrange("(g p) c -> p g c", p=P)
                    i = tb4[:, :gtiles * 2 * P] \
                        .rearrange("p (g c) -> p g c", c=2 * P)
                    nc.sync.dma_start(out=o, in_=i)
                else:
                    for g in range(gtiles):
                        rt = min(P, shard - (t0 + g) * P)
                        nc.sync.dma_start(
                            out=sh_t[(t0 + g) * P:(t0 + g) * P + rt, :],
                            in_=tb4[:rt, g * 2 * P:(g + 1) * 2 * P])

            def emit_ag(l):
                if "nocoll" in PROBE:
                    return
                nc.gpsimd.collective_compute(
                    "AllGather", mybir.AluOpType.bypass,
                    replica_groups=[list(range(NCORES))],
                    ins=[sh[l][:]], outs=[tbl[l][:]])

            # init the gather pool buffers once: trimmed gather calls leave
            # pad slots unwritten, and first-use SBUF could decode as NaN
            # (NaN * zero-m = NaN in the matmul)
            for kind_, es_ in (("s", es_s), ("g", es_g)):
                ktm = max(es_["Ktot"])
                for _ in range(GP_BUFS):
                    gz = gp.tile([P, ktm * P], f16, tag="gath" + kind_)
                    nc.vector.memset(gz[:], 0.0)

            # ---- layer 1 prologue: own-shard table + x0/linr/head ----
            for (t0, gtiles) in groups:
                xo4 = xp.tile([P, 4 * P], f16, tag="xo4")
                nc.sync.dma_start(out=xo4[:, :gtiles * P],
                                  in_=xto_d[:, t0 * P:(t0 + gtiles) * P])
                tb4 = bp.tile([P, 4 * 2 * P], f16, tag="tb4")
                for g in range(gtiles):
                    t = t0 + g
                    sl = slice(t * P, (t + 1) * P)
                    p1 = pq.tile([P, P], f32, tag="pa")
                    nc.tensor.matmul(p1[:], lhsT=wbt("fc1"),
                                     rhs=xo4[:, g * P:(g + 1) * P],
                                     start=True, stop=True)
                    nc.scalar.activation(x0T[:, sl], p1[:], Act.Relu,
                                         bias=wfc("fc1_b"))
                    ps = pp.tile([P, P], f32, tag="tbs")
                    nc.tensor.matmul(ps[:], lhsT=x0T[:, sl], rhs=wbt("sWl1"),
                                     start=True, stop=True)
                    pg = pp.tile([P, P], f32, tag="tbg")
                    nc.tensor.matmul(pg[:], lhsT=x0T[:, sl], rhs=wbt("gW1"),
                                     start=True, stop=True)
                    nc.scalar.activation(tb4[:, g * 2 * P:g * 2 * P + P],
                                         ps[:], Act.Copy)
                    nc.scalar.activation(tb4[:, g * 2 * P + P:(g + 1) * 2 * P],
                                         pg[:], Act.Copy,
                                         scale=rso_s[:, t:t + 1])
                write_sh(sh[0], t0, gtiles, tb4)

            if use_barrier:
                tc.strict_bb_all_engine_barrier()
            emit_ag(0)
            # linr/head work is independent of the collective — issued after
            # it so it executes during the AllGather instead of gating it
            for t in range(nt):
                sl = slice(t * P, (t + 1) * P)
                plr = pp.tile([P, GW * P], f32, tag="plr")
                nc.tensor.matmul(plr[:, :P], lhsT=wbt("sWr1"), rhs=x0T[:, sl],
                                 start=True, stop=False)
                nc.tensor.matmul(plr[:, :P], lhsT=ident_s[:], rhs=x0T[:, sl],
                                 start=False, stop=True)
                nc.scalar.activation(
                    linrg[t // GW][:, (t % GW) * P:(t % GW + 1) * P],
                    plr[:, :P], Act.Identity, bias=wfc("s_bl1"))
                ph = pr.tile([P, 1], f32, tag="ph")
                nc.tensor.matmul(ph[:], lhsT=x0T[:, sl],
                                 rhs=wh_s[:, 0:1], start=True, stop=True)
                nc.scalar.activation(hst[:, t:t + 1], ph[:], Act.Copy)
            if use_barrier:
                tc.strict_bb_all_engine_barrier()

            # ---- conv layers ----
            qctr = [0]

            def conv_group(kind, gi, W, tbl_t):
                """Aggregate one dst-group (W = gtiles*128 nodes) of one edge
                set into a [128, W] psum via per-region gathers + 512-wide
                one-hot matmuls."""
                es = es_s if kind == "s" else es_g
                idx_sb = idxs_s if kind == "s" else idxg_s
                dw = dws_s if kind == "s" else dwg_s
                SK = es["SK"]
                nreg = es["nreg"]
                colofs = 0 if kind == "s" else P
                pa = pq.tile([P, GW * P], f32, tag="pa")
                Kt = es["Ktot"][gi]
                off = es["coff"][gi][0]
                g = gp.tile([P, Kt * P], f16, tag="gath" + kind)
                if "nogather" in PROBE:
                    nc.vector.memset(g[:], 0.125)
                else:
                    for r in range(nreg):
                        kr = es["K"][gi][r]
                        cv = es["CV"][gi][r]
                        io = es["ioff"][gi][r]
                        co = es["coff"][gi][r] - off
                        for s in range(0, kr, CAPK):
                            kk = min(CAPK, kr - s)
                            vcall = min(kk * P, cv - s * P)
                            if vcall <= 0:
                                continue  # all-pad tail: zero-m, never read
                            qctr[0] += 1
                            nc.gpsimd.dma_gather(
                                out_ap=g[:, (co + s) * P:(co + s + kk) * P]
                                .rearrange("p (k e) -> p k e", e=P),
                                in_ap=tbl_t[bnd[r]:bnd[r + 1],
                                            colofs:colofs + P],
                                idxs_ap=idx_sb[:, io + s * 8:io + (s + kk) * 8],
                                num_idxs=vcall,
                                num_idxs_reg=vcall,
                                elem_size=P,
                                elem_step=2 * P,
                                single_packet=SPKT,
                                queue_num=qctr[0] % NQ)
                # whole-group one-hot build: 2 wide DVE ops; dl/wv columns are
                # stride-0 broadcast along the dst axis (layer-independent)
                m = mp.tile([P, KTM[kind], P], f16, tag="m" + kind)
                if "nom" in PROBE:
                    nc.vector.memset(m[:, :Kt, :W], 0.125)
                else:
                    dlb = dw[:, off:off + Kt].broadcast_to([P, Kt, W])
                    wvb = dw[:, SK + off:SK + off + Kt] \
                        .broadcast_to([P, Kt, W])
                    nc.vector.tensor_tensor(out=m[:, :Kt, :W],
                                            in0=iota_s[:, :Kt, :W],
                                            in1=dlb, op=Alu.is_equal)
                    nc.vector.tensor_tensor(out=m[:, :Kt, :W],
                                            in0=m[:, :Kt, :W],
                                            in1=wvb, op=Alu.mult)
                # accumulation group stays OPEN: the caller closes it with
                # identity-matmul residual adds (PE-side fused finalize)
                for k in range(Kt):
                    if "nomm" in PROBE and k > 0:
                        continue
                    nc.tensor.matmul(pa[:, :W], lhsT=g[:, k * P:(k + 1) * P],
                                     rhs=m[:, k, :W], start=(k == 0),
                                     stop=False)
                return pa

            for l in (1, 2, 3):
                tbl_t = tbl[l - 1][:]
                sh_next = sh[l] if l < 3 else None
                for gi, (t0, gtiles) in enumerate(groups):
                    W = gtiles * P
                    gsl = slice(t0 * P, t0 * P + W)
                    if l < 3:
                        tb4 = bp.tile([P, 4 * 2 * P], f16, tag="tb4")
                    else:
                        tb4 = None
                    # both conv chains BEFORE either finalize: the finalize
                    # waits on its psum (gathers+matmuls), and the in-order
                    # DVE queue would otherwise stall the second conv's
                    # one-hot builds behind it
                    # residual adds ride the PE as identity matmuls appended
                    # to each conv's accumulation group (groups stay
                    # contiguous per psum tile); evacuations ride ACT with
                    # fused bias. DVE only does head accumulation here.
                    pa = conv_group("s", gi, W, tbl_t)
                    # ocf = agg + linr (linr includes bl + x0 (+x1a))
                    nc.tensor.matmul(pa[:, :W], lhsT=ident_s[:],
                                     rhs=linrg[gi][:], start=False, stop=True)
                    pg = conv_group("g", gi, W, tbl_t)
                    # oaf = agg + x0 (+x1b) + g_b
                    nc.tensor.matmul(pg[:, :W], lhsT=ident_s[:],
                                     rhs=x0T[:, gsl], start=False,
                                     stop=(l == 1))
                    if l > 1:
                        nc.tensor.matmul(pg[:, :W], lhsT=ident_s[:],
                                         rhs=x1bT[:, gsl], start=False,
                                         stop=True)
                    if l == 1:
                        ocf = x1aT[:, gsl]
                        oaf = x1bT[:, gsl]
                    else:
                        ocf_t = op.tile([P, 4 * P], f16, tag="ocf")
                        ocf = ocf_t[:, :W]
                        oaf_t = op.tile([P, 4 * P], f16, tag="oaf")
                        oaf = oaf_t[:, :W]
                    nc.scalar.activation(ocf, pa[:, :W], Act.Copy)
                    nc.scalar.activation(oaf, pg[:, :W], Act.Identity,
                                         bias=wfc(f"g_b{l}"))
                    # heads on out_{l+1} (per tile: lhsT free dim <= 128)
                    hc = 1 + 2 * (l - 1)
                    for j in range(gtiles):
                        t = t0 + j
                        jsl = slice(j * P, (j + 1) * P)
                        ph = pr.tile([P, 1], f32, tag="ph")
                        nc.tensor.matmul(ph[:], lhsT=ocf[:, jsl],
                                         rhs=wh_s[:, hc:hc + 1],
                                         start=True, stop=False)
                        nc.tensor.matmul(ph[:], lhsT=oaf[:, jsl],
                                         rhs=wh_s[:, hc + 1:hc + 2],
                                         start=False, stop=True)
                        nc.scalar.activation(hst[:, l * nt + t:l * nt + t + 1],
                                             ph[:], Act.Copy)
                    if l == 3:
                        continue
                    # ---- boundary: tables (per tile) + linr (per group) ----
                    ln = l + 1
                    for j in range(gtiles):
                        t = t0 + j
                        jsl = slice(j * P, (j + 1) * P)
                        ps = pp.tile([P, P], f32, tag="tbs")
                        nc.tensor.matmul(ps[:], lhsT=ocf[:, jsl],
                                         rhs=wbt(f"sWl{ln}", 0),
                                         start=True, stop=False)
                        nc.tensor.matmul(ps[:], lhsT=oaf[:, jsl],
                                         rhs=wbt(f"sWl{ln}", 1),
                                         start=False, stop=True)
                        pgt = pp.tile([P, P], f32, tag="tbg")
                        nc.tensor.matmul(pgt[:], lhsT=ocf[:, jsl],
                                         rhs=wbt(f"gW{ln}", 0),
                                         start=True, stop=False)
                        nc.tensor.matmul(pgt[:], lhsT=oaf[:, jsl],
                                         rhs=wbt(f"gW{ln}", 1),
                                         start=False, stop=True)
                        nc.scalar.activation(tb4[:, j * 2 * P:j * 2 * P + P],
                                             ps[:], Act.Copy)
                        nc.scalar.activation(
                            tb4[:, j * 2 * P + P:(j + 1) * 2 * P],
                            pgt[:], Act.Copy, scale=rso_s[:, t:t + 1])
                    plr = pp.tile([P, GW * P], f32, tag="plr")
                    nc.tensor.matmul(plr[:, :W], lhsT=wbt(f"sWr{ln}", 0),
                                     rhs=ocf, start=True, stop=False)
                    nc.tensor.matmul(plr[:, :W], lhsT=wbt(f"sWr{ln}", 1),
                                     rhs=oaf, start=False, stop=False)
                    nc.tensor.matmul(plr[:, :W], lhsT=ident_s[:],
                                     rhs=x0T[:, gsl], start=False, stop=False)
                    nc.tensor.matmul(plr[:, :W], lhsT=ident_s[:],
                                     rhs=x1aT[:, gsl], start=False, stop=True)
                    nc.scalar.activation(linrg[gi][:], plr[:, :W],
                                         Act.Identity, bias=wfc(f"s_bl{ln}"))
                    write_sh(sh_next, t0, gtiles, tb4)
                if l < 3:
                    if use_barrier:
                        tc.strict_bb_all_engine_barrier()
                    emit_ag(l)
                    if use_barrier:
                        tc.strict_bb_all_engine_barrier()

            # ---- output ----
            nc.vector.tensor_tensor(out=resb[:], in0=hst[:, 0:nt],
                                    in1=hst[:, nt:2 * nt], op=Alu.add)
            nc.vector.tensor_tensor(out=resb[:], in0=resb[:],
                                    in1=hst[:, 2 * nt:3 * nt], op=Alu.add)
            nc.vector.scalar_tensor_tensor(
                out=resb[:], in0=hst[:, 3 * nt:4 * nt],
                scalar=float(meta["total_bias"]), in1=resb[:],
                op0=Alu.add, op1=Alu.add)
            nc.sync.dma_start(out=res_d[:], in_=resb[:])
        _stack.close()

    nc.compile()
    return nc


# ----------------------------------------------------------------------------
# entry point
# ----------------------------------------------------------------------------

def _run_and_bench(nc, in_maps, iters):
    """Mirror bass2jax.run_bass_via_pjrt's multi-core path, plus an optional
    pipelined repeat loop to measure marginal per-execution device time."""
    import time
    import jax
    import numpy as np
    from jax.sharding import Mesh, PartitionSpec
    from jax.experimental.shard_map import shard_map
    import concourse.mybir as mybir
    from concourse import bass2jax

    bass2jax.install_neuronx_cc_hook()
    partition_name = (nc.partition_id_tensor.name
                      if nc.partition_id_tensor else None)
    in_names, out_names, out_avals, zero_outs = [], [], [], []
    for alloc in nc.m.functions[0].allocations:
        if not isinstance(alloc, mybir.MemoryLocationSet):
            continue
        name = alloc.memorylocations[0].name
        if alloc.kind == "ExternalInput":
            if name != partition_name:
                in_names.append(name)
        elif alloc.kind == "ExternalOutput":
            shape = tuple(alloc.tensor_shape)
            dtype = mybir.dt.np(alloc.dtype)
            out_names.append(name)
            out_avals.append(jax.core.ShapedArray(shape, dtype))
            zero_outs.append(np.zeros(shape, dtype))
    n_params = len(in_names)
    all_in_names = list(in_names) + out_names
    if partition_name is not None:
        all_in_names.append(partition_name)

    def _body(*args):
        operands = list(args)
        if partition_name is not None:
            operands.append(bass2jax.partition_id_tensor())
        outs = bass2jax._bass_exec_p.bind(
            *operands, out_avals=tuple(out_avals),
            in_names=tuple(all_in_names), out_names=tuple(out_names),
            lowering_input_output_aliases=(),
            sim_require_finite=True, sim_require_nnan=True, nc=nc)
        return tuple(outs)

    devices = jax.devices()[:NCORES]
    mesh = Mesh(np.asarray(devices), ("core",))
    in_specs = (PartitionSpec("core"),) * (n_params + len(out_names))
    out_specs = (PartitionSpec("core"),) * len(out_names)
    sharded = jax.jit(shard_map(_body, mesh=mesh, in_specs=in_specs,
                                out_specs=out_specs, check_rep=False),
                      keep_unused=True)
    concat_in = [
        np.concatenate([np.asarray(in_maps[c][nm]) for c in range(NCORES)], 0)
        for nm in in_names]
    concat_zeros = [np.zeros((NCORES * z.shape[0], *z.shape[1:]), z.dtype)
                    for z in zero_outs]
    out_arrs = sharded(*concat_in, *concat_zeros)
    jax.block_until_ready(out_arrs)

    per_exec_ns = None
    if iters > 0:
        from jax.sharding import NamedSharding
        dev_in = [jax.device_put(a, NamedSharding(mesh, PartitionSpec("core")))
                  for a in concat_in]
        dev_zero = [jax.device_put(z, NamedSharding(mesh, PartitionSpec("core")))
                    for z in concat_zeros]
        r = sharded(*dev_in, *dev_zero)
        jax.block_until_ready(r)
        batches = int(os.environ.get("GNN_BATCHES", "10"))
        best = None
        for _ in range(batches):
            t1 = time.perf_counter()
            rs = [sharded(*dev_in, *dev_zero) for _ in range(iters)]
            jax.block_until_ready(rs)
            t2 = time.perf_counter()
            cur = (t2 - t1) / iters * 1e9
            best = cur if best is None else min(best, cur)
        per_exec_ns = best

    results = [
        {nm: np.asarray(out_arrs[i]).reshape(NCORES, *out_avals[i].shape)[c]
         for i, nm in enumerate(out_names)}
        for c in range(NCORES)]
    return results, per_exec_ns


def kernel(**inputs):
    global LAST_EXEC_NS, LAST_TRACE

    meta, in_maps = _prep(inputs)
    nc = _build(meta)

    iters = int(os.environ.get("GNN_BENCH", "0"))
    results, per_exec_ns = _run_and_bench(nc, in_maps, iters)
    LAST_EXEC_NS = per_exec_ns
    LAST_TRACE = None

    n, shard, nt = meta["n"], meta["shard"], meta["nt"]
    out = np.empty((n, 1), np.float32)
    for c in range(NCORES):
        r = results[c]["res"]  # [128, nt]
        out[c * shard:(c + 1) * shard, 0] = r.T.reshape(-1)[:shard]
    return out



# revision 48
# speedup vs baseline: 2.4483x; 2.2359x over previous
"""Trainium2 Bass kernel for a 3-layer SAGE+GCN GNN on 50k nodes / 800k edges,
sharded across 8 NeuronCores.

Strategy (v2):
  - Nodes sharded into 8 contiguous ranges; edges assigned to the core that
    owns their dst node (host sorts edges by dst tile / src region).
  - Per conv, inputs are pre-projected to 128-dim fp16 "table" rows
    ([node, 256] = sage half | gcn half); every layer's table is built from
    the core's own shard and AllGathered (layer 1 included — no replicated
    full-graph projection).
  - Aggregation: indirect-DMA gather of each dst-tile's edge rows into SBUF,
    then one-hot matmul segment-sum on TensorE (M[e,d] = (dst[e]==d) * w_e
    built on VectorE; w_e folds the SAGE 1/deg or GCN norm).
  - idx16 gather indices are persistent in SBUF (loaded once), removing
    per-gather DMA dispatches.
  - Everything node-indexed on-chip is feature-major ([feat, node]); psum
    evacuations ride the Activation engine; residual adds on VectorE in fp16.

Measured-on-HW facts that shaped this (timing probes, 2026-08):
  - dma_gather is ~100% of device time; spreading gathers round-robin over
    num_swdge_queues=4 (queue_num=) gave ~3x on the gather phase. One queue
    runs ~60 GB/s effective; 4 queues ~119 GB/s (close to the HBM
    random-256B-row wall).
  - A single gather call is capped by the 1024-entry descriptor ring
    (CAPK=8 chunks x 128 rows works; larger crashes NRT).
  - Pool/GPSIMD ALU ops cost ~3 us each on HW — never offload elementwise
    work there (POOL_M stays 0).
  - Real AllGather (3.2MB in / 25.6MB out, 8 cores) is ~70-90 us, 4x faster
    than the cost model claims; barriers around collectives are unnecessary
    (Tile deps suffice) and removed.
  - fp16 tables/weights (vs bf16) cost nothing and cut rel err ~5x.
  - num_idxs need not be a multiple of 128: trimming each cell's final
    gather call to the true 16-rounded row count drops the ~12% pad
    descriptors (requires the startup memsets of the gather pool buffers —
    unwritten slots are zero-m'd but NaN*0=NaN would poison the psum).
  - In-cell src-address sorting and single_packet=0 measured neutral: the
    gather wall is descriptor-processing rate, not DRAM row locality.
"""

import os
import numpy as np

P = 128
NCORES = 8

LAST_EXEC_NS = None
LAST_TRACE = None


# ----------------------------------------------------------------------------
# host-side preprocessing
# ----------------------------------------------------------------------------

CAPK = int(os.environ.get("GNN_CAPK", "8"))    # chunks per dma_gather call
GW = int(os.environ.get("GNN_GW", "1"))        # dst tiles per one-hot group


def _edge_streams(src, dst, w_edge, n, shard, nt, tw, perm, bnd):
    """Per-core gather/M-build streams for one edge set, organized for
    nc.gpsimd.dma_gather: per (dst-group g of tw nodes, src-region r)
    gathers of NV[g][r] valid rows (idx-0 padded to a cross-core-uniform
    count). dst-local offsets (dl) are relative to the group base
    (0..tw-1, exact in fp16 for tw<=2048).

    Regions are defined in PERMUTED (chunk-major) table space via `bnd`
    boundaries so each region's table slice is produced by its own
    chunked AllGather (perm[n] = permuted row of node n).

    Returns per-core (idx16 [128, SIC], dw [128, 2*SK]) plus layout lists.
    """
    nreg = len(bnd) - 1
    ngr = (nt * P + tw - 1) // tw
    percore = []
    counts = np.zeros((NCORES, ngr, nreg), np.int64)
    for c in range(NCORES):
        lo, hi = c * shard, (c + 1) * shard
        m = (dst >= lo) & (dst < hi)
        s_c, d_c, w_c = perm[src[m]], dst[m] - lo, w_edge[m]
        reg_c = np.searchsorted(bnd, s_c, side="right") - 1
        # by (group, region), then ascending src address within the cell —
        # the in-cell order is free (dl/wv follow it) and sorted addresses
        # give the DMA engines DRAM row/bank locality on the random reads
        order = np.lexsort((s_c, reg_c, d_c // tw))
        s_c, d_c, w_c, reg_c = s_c[order], d_c[order], w_c[order], reg_c[order]
        key = (d_c // tw) * nreg + reg_c
        bounds = np.searchsorted(key, np.arange(ngr * nreg + 1))
        counts[c] = (bounds[1:] - bounds[:-1]).reshape(ngr, nreg)
        percore.append((s_c, d_c, w_c, bounds))
    NV = counts.max(axis=0)                       # [ngr, nreg] max real count
    K = np.maximum((NV + P - 1) // P, 1)          # chunks per (g, r)
    # CV: true gather row count per cell, 16-rounded — the final gather call
    # of a cell is trimmed to this (pad slots beyond it are never fetched;
    # their m columns are zero so stale SBUF data cannot contribute)
    CV = np.minimum((NV + 15) // 16 * 16, K * P)
    NV = K * P                                    # all slots valid (idx-0 pads)
    Ktot = K.sum(axis=1)                          # chunks per group
    SK = int(Ktot.sum())
    coff = np.zeros((ngr, nreg), np.int64)
    ioff = np.zeros((ngr, nreg), np.int64)
    acc_c = 0
    acc_i = 0
    for t in range(ngr):
        for r in range(nreg):
            coff[t, r] = acc_c
            ioff[t, r] = acc_i
            acc_c += K[t, r]
            acc_i += K[t, r] * 8
    SIC = int(acc_i)

    outs = []
    for c in range(NCORES):
        s_c, d_c, w_c, bounds = percore[c]
        idx16 = np.zeros((P, SIC), np.int16)
        dl = np.full((P, SK), -1.0, np.float32)
        wv = np.zeros((P, SK), np.float32)
        for t in range(ngr):
            for r in range(nreg):
                b0, b1 = bounds[t * nreg + r], bounds[t * nreg + r + 1]
                cnt = b1 - b0
                kr = int(K[t, r])
                slots = kr * P
                buf_i = np.zeros(slots, np.int32)   # idx-0 pads (always write)
                buf_i[:cnt] = s_c[b0:b1] - bnd[r]
                buf_d = np.full(slots, -1.0, np.float32)
                buf_d[:cnt] = (d_c[b0:b1] - t * tw).astype(np.float32)
                buf_w = np.zeros(slots, np.float32)
                buf_w[:cnt] = w_c[b0:b1]
                # idx16: wrapped over 16 partitions, replicated x8
                cols = kr * 8
                wrap = buf_i.reshape(cols, 16).T.astype(np.int16)  # [16, cols]
                io = int(ioff[t, r])
                idx16[:, io:io + cols] = np.tile(wrap, (8, 1))
                # dl/wv: slot i -> partition i%128, chunk coff+i//128
                co = int(coff[t, r])
                dl[:, co:co + kr] = buf_d.reshape(kr, P).T
                wv[:, co:co + kr] = buf_w.reshape(kr, P).T
        # host-precomputed one-hot m tiles, weights folded (fp16): the DVE
        # is_equal build was the sim-trace bottleneck (97% busy); streaming
        # m from DRAM moves that work to idle DMA bandwidth
        mtab = np.zeros((P, SK, P), np.float16)
        dli = dl.astype(np.int64)
        valid = dli >= 0
        pi, ki = np.nonzero(valid)
        mtab[pi, ki, dli[pi, ki]] = wv[pi, ki]
        outs.append((idx16, mtab.reshape(P, SK * P)))
    meta_es = dict(K=K.tolist(), NV=NV.tolist(), CV=CV.tolist(),
                   Ktot=[int(x) for x in Ktot],
                   coff=coff.tolist(), ioff=ioff.tolist(),
                   SK=SK, SIC=SIC, nreg=nreg, ngr=ngr)
    return outs, meta_es


def _prep(inputs):
    inp = {k: np.asarray(v) for k, v in inputs.items()}
    x = inp["x"].astype(np.float32)
    n, din = x.shape
    assert din == P
    shard = n // NCORES
    nt = (shard + P - 1) // P

    src = inp["edge_index"][0].astype(np.int64)
    dst = inp["edge_index"][1].astype(np.int64)
    srca = inp["edge_index_aux"][0].astype(np.int64)
    dsta = inp["edge_index_aux"][1].astype(np.int64)

    deg = np.zeros(n, np.float32)
    np.add.at(deg, dst, 1.0)
    recip_deg = (1.0 / np.maximum(deg, 1.0)).astype(np.float32)
    dega = np.zeros(n, np.float32)
    np.add.at(dega, dsta, 1.0)
    deg_hat = dega + 1.0
    rs = (1.0 / np.sqrt(deg_hat)).astype(np.float32)

    # regions are int16 idx windows into the (unpermuted) table; a split
    # chunk-major AllGather was tried and measured WORSE (each collective
    # carries a ~180us fixed/barrier cost, so 6 AGs lose to 3)
    nregions = int(os.environ.get("GNN_NREG", "3"))
    bnd = np.linspace(0, n, nregions + 1).astype(np.int64)
    perm = np.arange(n, dtype=np.int64)
    assert int(np.diff(bnd).max()) < 32768  # int16 gather offsets

    sage_streams, es_s = _edge_streams(src, dst, recip_deg[dst], n, shard, nt,
                                       GW * P, perm, bnd)
    # GCN self-loop == a (i,i) edge with the same w = rs[dst] form, so fold it
    # into the edge stream (gather row i of the gcn half, scaled by rs[i])
    allnodes = np.arange(n, dtype=np.int64)
    srca_x = np.concatenate([srca, allnodes])
    dsta_x = np.concatenate([dsta, allnodes])
    gcn_streams, es_g = _edge_streams(srca_x, dsta_x, rs[dsta_x], n, shard, nt,
                                      GW * P, perm, bnd)

    f16 = np.float16

    # packed fp16 weights [P, 16*128]
    def w2(a):  # [d,128] -> list of [128,128] tiles
        a = np.asarray(a, np.float32)
        return [a[i * P:(i + 1) * P] for i in range(a.shape[0] // P)]

    wb_tiles = []
    wb_off = {}

    def put_b(name, tiles):
        wb_off[name] = len(wb_tiles) * P
        wb_tiles.extend(tiles)

    put_b("fc1", w2(inp["fc1_W"]))
    for l in (1, 2, 3):
        put_b(f"sWl{l}", w2(inp[f"s{l}_Wl"]))
        put_b(f"gW{l}", w2(inp[f"g{l}_W"]))
        put_b(f"sWr{l}", w2(inp[f"s{l}_Wr"]))
    wb = np.concatenate(wb_tiles, axis=1).astype(f16)  # [128, 16*128]

    # packed fp32 consts [P, ncols]
    wf_cols = []
    wf_off = {}

    def put_f(name, cols):
        a = np.asarray(cols, np.float32).reshape(-1)
        wf_off[name] = len(wf_cols)
        for i in range(a.shape[0] // P):
            wf_cols.append(a[i * P:(i + 1) * P])

    put_f("fc1_b", inp["fc1_b"])
    for l in (1, 2, 3):
        put_f(f"s_bl{l}", inp[f"s{l}_bl"])
        put_f(f"g_b{l}", inp[f"g{l}_b"])
    wf = np.stack(wf_cols, axis=1).astype(np.float32)  # [128, ncols]

    # fp16 head columns (scaled by w_i): [h1 | h2a h2b | h3a h3b | h4a h4b]
    w_scal = [float(inp[f"w{i}"][0]) for i in range(1, 5)]
    wh_cols = [inp["l1_W"].reshape(-1) * w_scal[0]]
    for i in (2, 3, 4):
        hw_ = inp[f"l{i}_W"].reshape(-1) * w_scal[i - 1]
        wh_cols.append(hw_[:P])
        wh_cols.append(hw_[P:])
    wh = np.stack(wh_cols, axis=1).astype(f16)  # [128, 7]
    total_bias = float(sum(float(inp[f"l{i}_b"][0]) * w_scal[i - 1]
                           for i in range(1, 5)))

    ktmx = max(max(es_s["Ktot"]), max(es_g["Ktot"]))

    meta = dict(n=n, shard=shard, nt=nt, ktmx=ktmx,
                bnd=[int(b) for b in bnd],
                es_s=es_s, es_g=es_g,
                wb_off=wb_off, wf_off=wf_off, wf_cols=wf.shape[1],
                total_bias=total_bias)

    in_maps = []
    for c in range(NCORES):
        lo = c * shard
        nown = min(shard, n - lo)
        ownx = np.zeros((nt * P, P), np.float32)
        ownx[:nown] = x[lo:lo + nown]
        # feature-major own x: xto[f, t*128+j] = x_own[t*128+j, f]
        xto = np.ascontiguousarray(
            ownx.reshape(nt, P, P).transpose(2, 0, 1).reshape(P, nt * P)
        ).astype(f16)
        rso = np.ones(nt * P, np.float32)
        rso[:nown] = rs[lo:lo + nown]
        idx_s, m_s = sage_streams[c]
        idx_g, m_g = gcn_streams[c]
        in_maps.append({
            "xto": xto,
            "idxs": idx_s, "ms": m_s,
            "idxg": idx_g, "mg": m_g,
            "wb": wb, "wf": wf, "wh": wh,
            "ident": np.eye(P, dtype=f16),
            "rso": rso.reshape(nt, P).T.copy(),
        })
    return meta, in_maps


# ----------------------------------------------------------------------------
# device program
# ----------------------------------------------------------------------------

def _build(meta):
    import concourse.bacc as bacc
    import concourse.mybir as mybir
    import concourse.tile as tile

    dt = mybir.dt
    Alu = mybir.AluOpType
    Act = mybir.ActivationFunctionType

    n, shard, nt = (meta[k] for k in ("n", "shard", "nt"))
    es_s, es_g = meta["es_s"], meta["es_g"]
    SKs, SKg = es_s["SK"], es_g["SK"]
    SICs, SICg = es_s["SIC"], es_g["SIC"]
    wbo, wfo = meta["wb_off"], meta["wf_off"]

    use_barrier = bool(int(os.environ.get("GNN_BARRIER", "0")))
    POOL_M = int(os.environ.get("GNN_POOL_M", "0"))  # of 8 m-builds on Pool
    PROBE = os.environ.get("GNN_PROBE", "")  # timing-only, comma-separated:
    PROBE = set(p for p in PROBE.split(",") if p)  # nom|nogather|nomm|nocoll
    GP_BUFS = int(os.environ.get("GNN_GP", "4"))  # gather pool depth; startup memsets below must cover ALL bufs
    SPKT = bool(int(os.environ.get("GNN_SPKT", "1")))

    scratch = int(os.environ.get("GNN_SCRATCH", "16384"))
    NQ = int(os.environ.get("GNN_NQ", "4"))
    nc = bacc.Bacc("TRN2", target_bir_lowering=False, debug=False,
                   num_devices=NCORES, dynamic_dma_scratch_size=scratch,
                   num_swdge_queues=NQ)

    def din(name, shape, dtype):
        return nc.dram_tensor(name, shape, dtype, kind="ExternalInput")

    xto_d = din("xto", [P, nt * P], dt.float16)
    idxs_d = din("idxs", [P, SICs], dt.int16)
    ms_d = din("ms", [P, SKs * P], dt.float16)
    idxg_d = din("idxg", [P, SICg], dt.int16)
    mg_d = din("mg", [P, SKg * P], dt.float16)
    wb_d = din("wb", [P, 16 * P], dt.float16)
    wf_d = din("wf", [P, meta["wf_cols"]], dt.float32)
    wh_d = din("wh", [P, 7], dt.float16)
    ident_d = din("ident", [P, P], dt.float16)
    rso_d = din("rso", [P, nt], dt.float32)
    res_d = nc.dram_tensor("res", [P, nt], dt.float32, kind="ExternalOutput")

    with tile.TileContext(nc) as tc:
        import contextlib
        _stack = contextlib.ExitStack()
        _ppool = _stack.enter_context(tc.tile_pool(name="persist", bufs=1))
        _dpool = _stack.enter_context(
            tc.tile_pool(name="persistd", bufs=1, space="DRAM"))

        def tc_tile(shape, dtype, space="SBUF", addr_space="Local", name="t"):
            pool = _dpool if space == "DRAM" else _ppool
            return pool.tile(shape, dtype, tag=name, name=name,
                             addr_space=addr_space)

        f32, f16 = dt.float32, dt.float16
        # --- persistent SBUF ---
        x0T = tc_tile([P, nt * P], f16, name="x0T")
        x1aT = tc_tile([P, nt * P], f16, name="x1aT")
        x1bT = tc_tile([P, nt * P], f16, name="x1bT")
        ngr = (nt + GW - 1) // GW
        linrg = [tc_tile([P, min(GW, nt - GW * gi) * P], f16,
                         name=f"linrg{gi}") for gi in range(ngr)]
        resb = tc_tile([P, nt], f32, name="resb")
        # per-layer head columns, evacuated by ACT; summed once at the end
        hst = tc_tile([P, 4 * nt], f16, name="hst")
        wb_s = tc_tile([P, 16 * P], f16, name="wb_s")
        wf_s = tc_tile([P, meta["wf_cols"]], f32, name="wf_s")
        wh_s = tc_tile([P, 7], f16, name="wh_s")
        ident_s = tc_tile([P, P], f16, name="ident_s")
        rso_s = tc_tile([P, nt], f32, name="rso_s")
        idxs_s = tc_tile([P, SICs], dt.int16, name="idxs_s")
        idxg_s = tc_tile([P, SICg], dt.int16, name="idxg_s")

        # --- DRAM tables ---
        bnd = meta["bnd"]
        sh = [tc_tile([shard, 2 * P], f16, space="DRAM", name=f"sh{l}")
              for l in (1, 2, 3)]
        tbl = [tc_tile([n, 2 * P], f16, space="DRAM", addr_space="Shared",
                       name=f"tbl{l}") for l in (1, 2, 3)]

        for t_, d_ in ((wb_s, wb_d), (wf_s, wf_d), (wh_s, wh_d),
                       (ident_s, ident_d), (rso_s, rso_d),
                       (idxs_s, idxs_d), (idxg_s, idxg_d)):
            nc.sync.dma_start(out=t_[:], in_=d_[:])

        KTM = {"s": max(es_s["Ktot"]), "g": max(es_g["Ktot"])}
        MP_BUFS = int(os.environ.get("GNN_MP", "3"))
        with (
            tc.tile_pool(name="xp", bufs=3) as xp,
            tc.tile_pool(name="gp", bufs=GP_BUFS) as gp,
            tc.tile_pool(name="mp", bufs=MP_BUFS) as mp,
            tc.tile_pool(name="op", bufs=4) as op,
            tc.tile_pool(name="bp", bufs=3) as bp,
            tc.tile_pool(name="pp", bufs=1, space="PSUM") as pp,
            tc.tile_pool(name="pq", bufs=3, space="PSUM") as pq,
            tc.tile_pool(name="pr", bufs=2, space="PSUM") as pr,
        ):
            def wbt(name, half=0):  # weight tile [128,128]
                o = wbo[name] + half * P
                return wb_s[:, o:o + P]

            def wfc(name, half=0):  # const col [128,1]
                o = wfo[name] + half
                return wf_s[:, o:o + 1]

            # group boundaries for batched sh writes: full groups of GW tiles
            groups = []
            t0 = 0
            while t0 < nt:
                gtiles = min(GW, nt - t0)
                if (t0 + gtiles) * P > shard:   # tail tile handled alone
                    gtiles = 1 if gtiles == 1 else gtiles - 1
                groups.append((t0, gtiles))
                t0 += gtiles

            batch_sh = bool(int(os.environ.get("GNN_BATCH_SH", "1")))

            def write_sh(sh_t, t0, gtiles, tb4):
                rows = min(shard - t0 * P, gtiles * P)
                if rows == gtiles * P and batch_sh:
                    o = sh_t[t0 * P:t0 * P + rows, :] \
                        .rearrange("(g p) c -> p g c", p=P)
                    i = tb4[:, :gtiles * 2 * P] \
                        .rearrange("p (g c) -> p g c", c=2 * P)
                    nc.sync.dma_start(out=o, in_=i)
                else:
                    for g in range(gtiles):
                        rt = min(P, shard - (t0 + g) * P)
                        nc.sync.dma_start(
                            out=sh_t[(t0 + g) * P:(t0 + g) * P + rt, :],
                            in_=tb4[:rt, g * 2 * P:(g + 1) * 2 * P])

            def emit_ag(l):
                if "nocoll" in PROBE:
                    return
                nc.gpsimd.collective_compute(
                    "AllGather", mybir.AluOpType.bypass,
                    replica_groups=[list(range(NCORES))],
                    ins=[sh[l][:]], outs=[tbl[l][:]])

            # init the gather pool buffers once: trimmed gather calls leave
            # pad slots unwritten, and first-use SBUF could decode as NaN
            # (NaN * zero-m = NaN in the matmul)
            for kind_, es_ in (("s", es_s), ("g", es_g)):
                ktm = max(es_["Ktot"])
                for _ in range(GP_BUFS):
                    gz = gp.tile([P, ktm * P], f16, tag="gath" + kind_)
                    nc.vector.memset(gz[:], 0.0)

            # ---- layer 1 prologue: own-shard table + x0/linr/head ----
            for (t0, gtiles) in groups:
                xo4 = xp.tile([P, 4 * P], f16, tag="xo4")
                nc.sync.dma_start(out=xo4[:, :gtiles * P],
                                  in_=xto_d[:, t0 * P:(t0 + gtiles) * P])
                tb4 = bp.tile([P, 4 * 2 * P], f16, tag="tb4")
                for g in range(gtiles):
                    t = t0 + g
                    sl = slice(t * P, (t + 1) * P)
                    p1 = pq.tile([P, P], f32, tag="pa")
                    nc.tensor.matmul(p1[:], lhsT=wbt("fc1"),
                                     rhs=xo4[:, g * P:(g + 1) * P],
                                     start=True, stop=True)
                    nc.scalar.activation(x0T[:, sl], p1[:], Act.Relu,
                                         bias=wfc("fc1_b"))
                    ps = pp.tile([P, P], f32, tag="tbs")
                    nc.tensor.matmul(ps[:], lhsT=x0T[:, sl], rhs=wbt("sWl1"),
                                     start=True, stop=True)
                    pg = pp.tile([P, P], f32, tag="tbg")
                    nc.tensor.matmul(pg[:], lhsT=x0T[:, sl], rhs=wbt("gW1"),
                                     start=True, stop=True)
                    nc.scalar.activation(tb4[:, g * 2 * P:g * 2 * P + P],
                                         ps[:], Act.Copy)
                    nc.scalar.activation(tb4[:, g * 2 * P + P:(g + 1) * 2 * P],
                                         pg[:], Act.Copy,
                                         scale=rso_s[:, t:t + 1])
                write_sh(sh[0], t0, gtiles, tb4)

            if use_barrier:
                tc.strict_bb_all_engine_barrier()
            emit_ag(0)
            # linr/head work is independent of the collective — issued after
            # it so it executes during the AllGather instead of gating it
            for t in range(nt):
                sl = slice(t * P, (t + 1) * P)
                plr = pp.tile([P, GW * P], f32, tag="plr")
                nc.tensor.matmul(plr[:, :P], lhsT=wbt("sWr1"), rhs=x0T[:, sl],
                                 start=True, stop=False)
                nc.tensor.matmul(plr[:, :P], lhsT=ident_s[:], rhs=x0T[:, sl],
                                 start=False, stop=True)
                nc.scalar.activation(
                    linrg[t // GW][:, (t % GW) * P:(t % GW + 1) * P],
                    plr[:, :P], Act.Identity, bias=wfc("s_bl1"))
                ph = pr.tile([P, 1], f32, tag="ph")
                nc.tensor.matmul(ph[:], lhsT=x0T[:, sl],
                                 rhs=wh_s[:, 0:1], start=True, stop=True)
                nc.scalar.activation(hst[:, t:t + 1], ph[:], Act.Copy)
            if use_barrier:
                tc.strict_bb_all_engine_barrier()

            # ---- conv layers ----
            qctr = [0]

            def conv_group(kind, gi, W, tbl_t):
                """Aggregate one dst-group (W = gtiles*128 nodes) of one edge
                set into a [128, W] psum via per-region gathers + 512-wide
                one-hot matmuls."""
                es = es_s if kind == "s" else es_g
                idx_sb = idxs_s if kind == "s" else idxg_s
                m_d = ms_d if kind == "s" else mg_d
                SK = es["SK"]
                nreg = es["nreg"]
                colofs = 0 if kind == "s" else P
                pa = pq.tile([P, GW * P], f32, tag="pa")
                Kt = es["Ktot"][gi]
                off = es["coff"][gi][0]
                g = gp.tile([P, Kt * P], f16, tag="gath" + kind)
                if "nogather" in PROBE:
                    nc.vector.memset(g[:], 0.125)
                else:
                    for r in range(nreg):
                        kr = es["K"][gi][r]
                        cv = es["CV"][gi][r]
                        io = es["ioff"][gi][r]
                        co = es["coff"][gi][r] - off
                        for s in range(0, kr, CAPK):
                            kk = min(CAPK, kr - s)
                            vcall = min(kk * P, cv - s * P)
                            if vcall <= 0:
                                continue  # all-pad tail: zero-m, never read
                            qctr[0] += 1
                            nc.gpsimd.dma_gather(
                                out_ap=g[:, (co + s) * P:(co + s + kk) * P]
                                .rearrange("p (k e) -> p k e", e=P),
                                in_ap=tbl_t[bnd[r]:bnd[r + 1],
                                            colofs:colofs + P],
                                idxs_ap=idx_sb[:, io + s * 8:io + (s + kk) * 8],
                                num_idxs=vcall,
                                num_idxs_reg=vcall,
                                elem_size=P,
                                elem_step=2 * P,
                                single_packet=SPKT,
                                queue_num=qctr[0] % NQ)
                # stream the host-built one-hot m tiles for this group
                m = mp.tile([P, KTM[kind], P], f16, tag="m" + kind)
                nc.sync.dma_start(
                    out=m[:, :Kt, :],
                    in_=m_d[:, off * P:(off + Kt) * P]
                    .rearrange("p (k e) -> p k e", e=P))
                # accumulation group stays OPEN: the caller closes it with
                # identity-matmul residual adds (PE-side fused finalize)
                for k in range(Kt):
                    if "nomm" in PROBE and k > 0:
                        continue
                    nc.tensor.matmul(pa[:, :W], lhsT=g[:, k * P:(k + 1) * P],
                                     rhs=m[:, k, :W], start=(k == 0),
                                     stop=False)
                return pa

            for l in (1, 2, 3):
                tbl_t = tbl[l - 1][:]
                sh_next = sh[l] if l < 3 else None
                for gi, (t0, gtiles) in enumerate(groups):
                    W = gtiles * P
                    gsl = slice(t0 * P, t0 * P + W)
                    if l < 3:
                        tb4 = bp.tile([P, 4 * 2 * P], f16, tag="tb4")
                    else:
                        tb4 = None
                    # both conv chains BEFORE either finalize: the finalize
                    # waits on its psum (gathers+matmuls), and the in-order
                    # DVE queue would otherwise stall the second conv's
                    # one-hot builds behind it
                    # residual adds ride the PE as identity matmuls appended
                    # to each conv's accumulation group (groups stay
                    # contiguous per psum tile); evacuations ride ACT with
                    # fused bias. DVE only does head accumulation here.
                    pa = conv_group("s", gi, W, tbl_t)
                    # ocf = agg + linr (linr includes bl + x0 (+x1a))
                    nc.tensor.matmul(pa[:, :W], lhsT=ident_s[:],
                                     rhs=linrg[gi][:], start=False, stop=True)
                    pg = conv_group("g", gi, W, tbl_t)
                    # oaf = agg + x0 (+x1b) + g_b
                    nc.tensor.matmul(pg[:, :W], lhsT=ident_s[:],
                                     rhs=x0T[:, gsl], start=False,
                                     stop=(l == 1))
                    if l > 1:
                        nc.tensor.matmul(pg[:, :W], lhsT=ident_s[:],
                                         rhs=x1bT[:, gsl], start=False,
                                         stop=True)
                    if l == 1:
                        ocf = x1aT[:, gsl]
                        oaf = x1bT[:, gsl]
                    else:
                        ocf_t = op.tile([P, 4 * P], f16, tag="ocf")
                        ocf = ocf_t[:, :W]
                        oaf_t = op.tile([P, 4 * P], f16, tag="oaf")
                        oaf = oaf_t[:, :W]
                    nc.scalar.activation(ocf, pa[:, :W], Act.Copy)
                    nc.scalar.activation(oaf, pg[:, :W], Act.Identity,
                                         bias=wfc(f"g_b{l}"))
                    # heads on out_{l+1} (per tile: lhsT free dim <= 128)
                    hc = 1 + 2 * (l - 1)
                    for j in range(gtiles):
                        t = t0 + j
                        jsl = slice(j * P, (j + 1) * P)
                        ph = pr.tile([P, 1], f32, tag="ph")
                        nc.tensor.matmul(ph[:], lhsT=ocf[:, jsl],
                                         rhs=wh_s[:, hc:hc + 1],
                                         start=True, stop=False)
                        nc.tensor.matmul(ph[:], lhsT=oaf[:, jsl],
                                         rhs=wh_s[:, hc + 1:hc + 2],
                                         start=False, stop=True)
                        nc.scalar.activation(hst[:, l * nt + t:l * nt + t + 1],
                                             ph[:], Act.Copy)
                    if l == 3:
                        continue
                    # ---- boundary: tables (per tile) + linr (per group) ----
                    ln = l + 1
                    for j in range(gtiles):
                        t = t0 + j
                        jsl = slice(j * P, (j + 1) * P)
                        ps = pp.tile([P, P], f32, tag="tbs")
                        nc.tensor.matmul(ps[:], lhsT=ocf[:, jsl],
                                         rhs=wbt(f"sWl{ln}", 0),
                                         start=True, stop=False)
                        nc.tensor.matmul(ps[:], lhsT=oaf[:, jsl],
                                         rhs=wbt(f"sWl{ln}", 1),
                                         start=False, stop=True)
                        pgt = pp.tile([P, P], f32, tag="tbg")
                        nc.tensor.matmul(pgt[:], lhsT=ocf[:, jsl],
                                         rhs=wbt(f"gW{ln}", 0),
                                         start=True, stop=False)
                        nc.tensor.matmul(pgt[:], lhsT=oaf[:, jsl],
                                         rhs=wbt(f"gW{ln}", 1),
                                         start=False, stop=True)
                        nc.scalar.activation(tb4[:, j * 2 * P:j * 2 * P + P],
                                             ps[:], Act.Copy)
                        nc.scalar.activation(
                            tb4[:, j * 2 * P + P:(j + 1) * 2 * P],
                            pgt[:], Act.Copy, scale=rso_s[:, t:t + 1])
                    plr = pp.tile([P, GW * P], f32, tag="plr")
                    nc.tensor.matmul(plr[:, :W], lhsT=wbt(f"sWr{ln}", 0),
                                     rhs=ocf, start=True, stop=False)
                    nc.tensor.matmul(plr[:, :W], lhsT=wbt(f"sWr{ln}", 1),
                                     rhs=oaf, start=False, stop=False)
                    nc.tensor.matmul(plr[:, :W], lhsT=ident_s[:],
                                     rhs=x0T[:, gsl], start=False, stop=False)
                    nc.tensor.matmul(plr[:, :W], lhsT=ident_s[:],
                                     rhs=x1aT[:, gsl], start=False, stop=True)
                    nc.scalar.activation(linrg[gi][:], plr[:, :W],
                                         Act.Identity, bias=wfc(f"s_bl{ln}"))
                    write_sh(sh_next, t0, gtiles, tb4)
                if l < 3:
                    if use_barrier:
                        tc.strict_bb_all_engine_barrier()
                    emit_ag(l)
                    if use_barrier:
                        tc.strict_bb_all_engine_barrier()

            # ---- output ----
            nc.vector.tensor_tensor(out=resb[:], in0=hst[:, 0:nt],
                                    in1=hst[:, nt:2 * nt], op=Alu.add)
            nc.vector.tensor_tensor(out=resb[:], in0=resb[:],
                                    in1=hst[:, 2 * nt:3 * nt], op=Alu.add)
            nc.vector.scalar_tensor_tensor(
                out=resb[:], in0=hst[:, 3 * nt:4 * nt],
                scalar=float(meta["total_bias"]), in1=resb[:],
                op0=Alu.add, op1=Alu.add)
            nc.sync.dma_start(out=res_d[:], in_=resb[:])
        _stack.close()

    nc.compile()
    return nc


# ----------------------------------------------------------------------------
# entry point
# ----------------------------------------------------------------------------

def _run_and_bench(nc, in_maps, iters):
    """Mirror bass2jax.run_bass_via_pjrt's multi-core path, plus an optional
    pipelined repeat loop to measure marginal per-execution device time."""
    import time
    import jax
    import numpy as np
    from jax.sharding import Mesh, PartitionSpec
    from jax.experimental.shard_map import shard_map
    import concourse.mybir as mybir
    from concourse import bass2jax

    bass2jax.install_neuronx_cc_hook()
    partition_name = (nc.partition_id_tensor.name
                      if nc.partition_id_tensor else None)
    in_names, out_names, out_avals, zero_outs = [], [], [], []
    for alloc in nc.m.functions[0].allocations:
        if not isinstance(alloc, mybir.MemoryLocationSet):
            continue
        name = alloc.memorylocations[0].name
        if alloc.kind == "ExternalInput":
            if name != partition_name:
                in_names.append(name)
        elif alloc.kind == "ExternalOutput":
            shape = tuple(alloc.tensor_shape)
            dtype = mybir.dt.np(alloc.dtype)
            out_names.append(name)
            out_avals.append(jax.core.ShapedArray(shape, dtype))
            zero_outs.append(np.zeros(shape, dtype))
    n_params = len(in_names)
    all_in_names = list(in_names) + out_names
    if partition_name is not None:
        all_in_names.append(partition_name)

    def _body(*args):
        operands = list(args)
        if partition_name is not None:
            operands.append(bass2jax.partition_id_tensor())
        outs = bass2jax._bass_exec_p.bind(
            *operands, out_avals=tuple(out_avals),
            in_names=tuple(all_in_names), out_names=tuple(out_names),
            lowering_input_output_aliases=(),
            sim_require_finite=True, sim_require_nnan=True, nc=nc)
        return tuple(outs)

    devices = jax.devices()[:NCORES]
    mesh = Mesh(np.asarray(devices), ("core",))
    in_specs = (PartitionSpec("core"),) * (n_params + len(out_names))
    out_specs = (PartitionSpec("core"),) * len(out_names)
    sharded = jax.jit(shard_map(_body, mesh=mesh, in_specs=in_specs,
                                out_specs=out_specs, check_rep=False),
                      keep_unused=True)
    concat_in = [
        np.concatenate([np.asarray(in_maps[c][nm]) for c in range(NCORES)], 0)
        for nm in in_names]
    concat_zeros = [np.zeros((NCORES * z.shape[0], *z.shape[1:]), z.dtype)
                    for z in zero_outs]
    out_arrs = sharded(*concat_in, *concat_zeros)
    jax.block_until_ready(out_arrs)

    per_exec_ns = None
    if iters > 0:
        from jax.sharding import NamedSharding
        dev_in = [jax.device_put(a, NamedSharding(mesh, PartitionSpec("core")))
                  for a in concat_in]
        dev_zero = [jax.device_put(z, NamedSharding(mesh, PartitionSpec("core")))
                    for z in concat_zeros]
        r = sharded(*dev_in, *dev_zero)
        jax.block_until_ready(r)
        batches = int(os.environ.get("GNN_BATCHES", "10"))
        best = None
        for _ in range(batches):
            t1 = time.perf_counter()
            rs = [sharded(*dev_in, *dev_zero) for _ in range(iters)]
            jax.block_until_ready(rs)
            t2 = time.perf_counter()
            cur = (t2 - t1) / iters * 1e9
            best = cur if best is None else min(best, cur)
        per_exec_ns = best

    results = [
        {nm: np.asarray(out_arrs[i]).reshape(NCORES, *out_avals[i].shape)[c]
         for i, nm in enumerate(out_names)}
        for c in range(NCORES)]
    return results, per_exec_ns


def kernel(**inputs):
    global LAST_EXEC_NS, LAST_TRACE

    meta, in_maps = _prep(inputs)
    nc = _build(meta)

    iters = int(os.environ.get("GNN_BENCH", "0"))
    results, per_exec_ns = _run_and_bench(nc, in_maps, iters)
    LAST_EXEC_NS = per_exec_ns
    LAST_TRACE = None

    n, shard, nt = meta["n"], meta["shard"], meta["nt"]
    out = np.empty((n, 1), np.float32)
    for c in range(NCORES):
        r = results[c]["res"]  # [128, nt]
        out[c * shard:(c + 1) * shard, 0] = r.T.reshape(-1)[:shard]
    return out

